# revision 32
# baseline (speedup 1.0000x reference)
"""ClusterGCN (3-layer GCN, sum-aggregation) on 8 Trainium2 NeuronCores.

Strategy (hardcoded for B=2, N=50000, F=H=128, E=800000, 8 cores):
  - core c: destination shard c (6250 nodes), BOTH batches. Tables are
    batch-interleaved: row n = [h(b0,n,:) | h(b1,n,:)] in bf16 (512B), so one
    gather index fetches both batches' source rows.
  - Reassociate each layer: A @ (h @ W) == (A @ h) @ W: aggregate first
    (segment-sum over edges), then one dense 128x128 matmul per batch.
  - Edges sorted by (dst_tile, ...) into 128-slot chunks; each chunk -> one
    is_equal one-hot [slot, dst_rel] and accumulating matmuls into PSUM agg
    tiles [feat, dst].
  - LAYER 0: x is a kernel input, so the gather is STATIC -> pre-gather x on
    the HOST into chunk-slot order and stream it with plain HWDGE DMA.
    Zero SWDGE descriptors (the Q7 descriptor-gen engine is the bottleneck
    at ~4.2ns/index, ~480us per gather layer).
  - LAYER 1: SWDGE dma_gather from the block-layout htab (bf16). Gather
    calls are <=1024 idx (hard Q7 scratch limit; 2048 wedges the device),
    cycled over SWDGE queues 0..3. The htab AllGather is split into 4
    tile-range quarters so the next layer's lo-bucket gathers start after
    quarters 0+1 land, and quarters 2+3 hide behind phase-A gathers.
  - LAYER 2: output feature width is 1, so compute s = relu(bn(h2)) @ W3
    per-shard BEFORE communicating: AllGather only [50000, 2] f32 (50KB/core)
    instead of the 3.2MB/core table, then expand locally into a
    [50048, 128] bf16 table (row n = (s_b0, s_b1) replicated 64x, 256B = the
    SWDGE gather minimum elem). Layer-2 gathers fetch 256B rows and
    aggregate with [slot,2] x [slot,dst] matmuls into [2, dst] PSUM tiles.
  - BatchNorm is training-mode over all B*N rows: per-core bn_stats/bn_aggr,
    then an 8-core AllReduce of (mean, E[x^2]).
  - SPMD: one instruction stream for all 8 cores -> call schedules are
    canonical (per-group max chunk count over shards); each shard pads its
    own chunks with idx 0 / dst_rel 255 (one-hot all-zero).
"""

import math

import numpy as np

P = 128
FEAT = 128
CPC = 8  # max chunks per gather call (8*128 = 1024 idx, Q7 scratch limit)

# htab quarter-block layout (tile ranges per shard; lo bucket = q0+q1)
QT = [0, 13, 25, 37, 49]             # tile boundaries of the 4 AG pieces
QSTART = [0, 13 * P, 25 * P, 37 * P]  # row starts within a shard
SSH = 49 * P                          # padded s_shard rows per core (6272)
S_PAD = 8 * SSH                       # s_tab rows (50176)
S_HALF = 4 * SSH                      # s_tab lo/hi bucket boundary (25088)
SW = 256                              # s_tab row width in bf16 (512B)
SLAB_R = 13                           # s_tab expansion rows per partition
SLAB_N = SLAB_R * P                   # nodes per expansion slab (1664)


class Cfg:
    def __init__(self, n_nodes=50000, batch=2, eps=1e-5):
        self.N = n_nodes
        self.SHARD = n_nodes // 8  # 6250
        self.BATCH = batch
        self.HALF = n_nodes // 2
        self.TILES = math.ceil(self.SHARD / P)  # 49
        self.VALID_LAST = self.SHARD - (self.TILES - 1) * P  # 106
        self.EPS = eps
        self.QSZ = [QSTART[1], QSTART[2] - QSTART[1], QSTART[3] - QSTART[2],
                    self.SHARD - QSTART[3]]
        self.QOFF = [8 * q for q in QSTART]  # block offsets in htab
        self.HHALF = 8 * QSTART[2]  # 25600: htab lo/hi gather view split


def _wrap16(stream):
    """[n] idx stream -> [128, n/16] wrapped col-major, replicated x8."""
    return np.tile(stream.reshape(-1, 16).T, (8, 1))


def build_schedule(cfg, edge_index, remap=None, half=None):
    """Canonical dst-sorted chunk schedule shared by all 8 shards.

    remap: optional vectorized fn mapping global source ids to table rows
    (used for the quarter-block htab layout of layer 1).

    Returns (calls, chunk_meta, wi_list, wd_list):
      calls: list of (bucket, chunk0, nch) gather calls
      chunk_meta: per chunk (tile, first_in_group, last_in_group)
      wi_list[q]: [128, n_chunks*8] i16 wrapped gather idx for shard q
      wd_list[q]: [128, n_chunks] f32 per-chunk dst_rel (along partitions)
    """
    row = np.asarray(edge_index[0]).astype(np.int64)
    col = np.asarray(edge_index[1]).astype(np.int64)
    if remap is not None:
        row = remap(row)
    if half is None:
        half = cfg.HALF

    groups = []
    for q in range(8):
        base = q * cfg.SHARD
        m = (col >= base) & (col < base + cfg.SHARD)
        r = row[m]
        c = col[m] - base
        t = c // P
        drel = c % P
        bkt = (r >= half).astype(np.int64)
        order = np.lexsort((r, drel, bkt, t))
        r, t, drel, bkt = r[order], t[order], drel[order], bkt[order]
        idx16 = np.where(bkt == 1, r - half, r).astype(np.int16)
        g = {}
        key = t * 2 + bkt
        bounds = np.flatnonzero(np.append(True, key[1:] != key[:-1]))
        bounds = np.append(bounds, len(key))
        for j in range(len(bounds) - 1):
            s, e = int(bounds[j]), int(bounds[j + 1])
            g[(int(t[s]), int(bkt[s]))] = (idx16[s:e], drel[s:e].astype(np.float32))
        groups.append(g)

    kmax = {}
    for t in range(cfg.TILES):
        for b in (0, 1):
            n = max(len(g.get((t, b), ((), ()))[0]) for g in groups)
            kmax[(t, b)] = max(1, math.ceil(n / P))

    chunk_of = {}
    chunk_meta = []
    c0 = 0
    for b in (0, 1):
        for t in range(cfg.TILES):
            k = kmax[(t, b)]
            chunk_of[(t, b)] = c0
            for j in range(k):
                chunk_meta.append((t, j == 0, j == k - 1))
            c0 += k
    nch_total = c0
    calls = []
    nlo = sum(kmax[(t, 0)] for t in range(cfg.TILES))
    for b, (lo, hi) in ((0, (0, nlo)), (1, (nlo, nch_total))):
        for s2 in range(lo, hi, CPC):
            calls.append((b, s2, min(CPC, hi - s2)))

    wi_list, wd_list = [], []
    for q in range(8):
        wi = np.zeros((128, nch_total * 8), np.int16)
        wd = np.full((128, nch_total), 255.0, np.float32)
        for (t, b), cc0 in chunk_of.items():
            idx16, drel = groups[q].get((t, b), (np.zeros(0, np.int16),
                                                 np.zeros(0, np.float32)))
            k = kmax[(t, b)]
            pi = np.zeros(k * P, np.int16)
            pd = np.full(k * P, 255.0, np.float32)
            pi[:len(idx16)] = idx16
            pd[:len(drel)] = drel
            wi[:, cc0 * 8:(cc0 + k) * 8] = _wrap16(pi)
            wd[:, cc0:cc0 + k] = pd.reshape(k, P).T
        wi_list.append(wi)
        wd_list.append(wd)
    return calls, chunk_meta, wi_list, wd_list


def build_schedule_x(cfg, edge_index):
    """Single-bucket canonical schedule for layer 0 (host pre-gather).

    Returns (calls, chunk_meta, src_list, wd_list):
      calls: list of (chunk0, nch)
      src_list[q]: [nch_total, 128] int64 global source ids per slot (pad 0)
      wd_list[q]: [128, nch_total] f32 dst_rel (pad 255)
    """
    row = np.asarray(edge_index[0]).astype(np.int64)
    col = np.asarray(edge_index[1]).astype(np.int64)
    groups = []
    for q in range(8):
        base = q * cfg.SHARD
        m = (col >= base) & (col < base + cfg.SHARD)
        r = row[m]
        c = col[m] - base
        t = c // P
        drel = c % P
        order = np.lexsort((r, drel, t))
        r, t, drel = r[order], t[order], drel[order]
        g = {}
        bounds = np.flatnonzero(np.append(True, t[1:] != t[:-1]))
        bounds = np.append(bounds, len(t))
        for j in range(len(bounds) - 1):
            s, e = int(bounds[j]), int(bounds[j + 1])
            g[int(t[s])] = (r[s:e], drel[s:e].astype(np.float32))
        groups.append(g)

    kmax = []
    for t in range(cfg.TILES):
        n = max(len(g.get(t, ((), ()))[0]) for g in groups)
        kmax.append(max(1, math.ceil(n / P)))
    chunk_meta = []
    for t in range(cfg.TILES):
        for j in range(kmax[t]):
            chunk_meta.append((t, j == 0, j == kmax[t] - 1))
    nch_total = len(chunk_meta)
    calls = [(s2, min(CPC, nch_total - s2)) for s2 in range(0, nch_total, CPC)]

    src_list, wd_list = [], []
    for q in range(8):
        srcs = np.zeros((nch_total, P), np.int64)
        wd = np.full((128, nch_total), 255.0, np.float32)
        c0 = 0
        for t in range(cfg.TILES):
            r, drel = groups[q].get(t, (np.zeros(0, np.int64),
                                        np.zeros(0, np.float32)))
            k = kmax[t]
            pr = np.zeros(k * P, np.int64)
            pd = np.full(k * P, 255.0, np.float32)
            pr[:len(r)] = r
            pd[:len(drel)] = drel
            srcs[c0:c0 + k] = pr.reshape(k, P)
            wd[:, c0:c0 + k] = pd.reshape(k, P).T
            c0 += k
        src_list.append(srcs)
        wd_list.append(wd)
    return calls, chunk_meta, src_list, wd_list


# ---------------------------------------------------------------- bass kernel


def build_nc(cfg, scheds, shapes):
    import concourse.bacc as bacc
    import concourse.bass as bass
    import concourse.tile as tile
    from concourse import mybir

    f32 = mybir.dt.float32
    bf16 = mybir.dt.bfloat16
    i16 = mybir.dt.int16
    TW = 2 * FEAT  # interleaved table width (256)

    (calls_x, meta_x), (calls_h, meta_h), (calls_s, meta_s) = scheds

    nc = bacc.Bacc("TRN2", target_bir_lowering=False, debug=False,
                   num_devices=8, num_swdge_queues=4)

    xg_d = nc.dram_tensor("xg", list(shapes["xg"]), bf16, kind="ExternalInput")
    wih_d = nc.dram_tensor("wih", list(shapes["wih"]), i16, kind="ExternalInput")
    wis_d = nc.dram_tensor("wis", list(shapes["wis"]), i16, kind="ExternalInput")
    wdx_d = nc.dram_tensor("wdx", list(shapes["wdx"]), bf16, kind="ExternalInput")
    wdh_d = nc.dram_tensor("wdh", list(shapes["wdh"]), bf16, kind="ExternalInput")
    wds_d = nc.dram_tensor("wds", list(shapes["wds"]), bf16, kind="ExternalInput")
    w_dr = [nc.dram_tensor(f"W{i+1}", [FEAT, FEAT if i < 2 else 1], f32,
                           kind="ExternalInput") for i in range(3)]
    b_dr = [nc.dram_tensor(f"b{i+1}", [FEAT if i < 2 else 1], f32,
                           kind="ExternalInput") for i in range(3)]
    gb_dr = [(nc.dram_tensor(f"gamma{i+1}", [FEAT], f32, kind="ExternalInput"),
              nc.dram_tensor(f"beta{i+1}", [FEAT], f32, kind="ExternalInput"))
             for i in range(2)]
    iota_p_d = nc.dram_tensor("iota_p", [P, P], bf16, kind="ExternalInput")
    ident_d = nc.dram_tensor("ident", [P, P], f32, kind="ExternalInput")
    out_d = nc.dram_tensor("out", [cfg.BATCH, cfg.SHARD], f32, kind="ExternalOutput")

    htab = nc.dram_tensor("htab0", [cfg.N, TW], bf16, kind="Internal",
                          addr_space="Shared")
    shard_out = nc.dram_tensor("shard_out0", [cfg.SHARD, TW], bf16,
                               kind="Internal")
    s_shard = nc.dram_tensor("s_shard", [SSH, 2], f32, kind="Internal")
    s_full = nc.dram_tensor("s_full", [S_PAD, 2], f32, kind="Internal")
    s_tab = nc.dram_tensor("s_tab", [S_PAD, SW], bf16, kind="Internal")
    stat_in = [nc.dram_tensor(f"stat_in{i}", [P, 2], f32, kind="Internal")
               for i in range(2)]
    stat_out = [nc.dram_tensor(f"stat_out{i}", [8 * P, 2], f32,
                               kind="Internal", addr_space="Shared")
                for i in range(2)]

    AluOp = mybir.AluOpType
    ActF = mybir.ActivationFunctionType

    def bcast_inner(ap, inner):
        return bass.AP(tensor=ap.tensor, offset=ap.offset,
                       ap=[list(ap.ap[0]), list(ap.ap[1]), [0, inner]])

    def bcast_rep(ap, reps):
        return bass.AP(tensor=ap.tensor, offset=ap.offset,
                       ap=[list(ap.ap[0]), [0, reps], list(ap.ap[1])])

    def re_ap(ap, free_dims, extra_off=0):
        """Same tensor/partition dim, custom free-dim APs."""
        return bass.AP(tensor=ap.tensor, offset=ap.offset + extra_off,
                       ap=[list(ap.ap[0])] + [list(d) for d in free_dims])

    with tile.TileContext(nc) as tc:
        with (
            tc.tile_pool(name="consts", bufs=1) as consts,
            tc.tile_pool(name="gw", bufs=8) as gwp,
            tc.tile_pool(name="ohp", bufs=6) as ohp,
            tc.tile_pool(name="aggp", bufs=4) as aggp,
            tc.tile_pool(name="hraw", bufs=1) as hrawp,
            tc.tile_pool(name="aglo", bufs=1) as aglop,
            tc.tile_pool(name="statp", bufs=2) as statp,
            tc.tile_pool(name="small", bufs=8) as small,
            tc.tile_pool(name="p2", bufs=6) as p2p,
            tc.tile_pool(name="sexp", bufs=2) as sexpp,
            tc.tile_pool(name="outp", bufs=1) as outp,
            tc.tile_pool(name="ps_agg", bufs=2, space="PSUM") as ps_agg,
            tc.tile_pool(name="ps_h", bufs=2, space="PSUM") as ps_h,
            tc.tile_pool(name="ps_t", bufs=2, space="PSUM") as ps_t,
        ):
            wih_sb = consts.tile(list(shapes["wih"]), i16, tag="wih")
            nc.sync.dma_start(out=wih_sb[:], in_=wih_d[:])
            wis_sb = consts.tile(list(shapes["wis"]), i16, tag="wis")
            nc.sync.dma_start(out=wis_sb[:], in_=wis_d[:])
            wdx_sb = consts.tile(list(shapes["wdx"]), bf16, tag="wdx")
            nc.sync.dma_start(out=wdx_sb[:], in_=wdx_d[:])
            wdh_sb = consts.tile(list(shapes["wdh"]), bf16, tag="wdh")
            nc.sync.dma_start(out=wdh_sb[:], in_=wdh_d[:])
            wds_sb = consts.tile(list(shapes["wds"]), bf16, tag="wds")
            nc.sync.dma_start(out=wds_sb[:], in_=wds_d[:])
            w_sb = []
            for i, wdr in enumerate(w_dr):
                t = consts.tile([P, FEAT if i < 2 else 1], f32, tag=f"w{i}")
                nc.sync.dma_start(out=t[:], in_=wdr[:])
                w_sb.append(t)
            b_sb = []
            for i in range(2):
                t = consts.tile([P, 1], f32, tag=f"b{i}")
                nc.sync.dma_start(out=t[:], in_=b_dr[i][:, None])
                b_sb.append(t)
            b3_2 = consts.tile([2, 1], f32, tag="b3_2")
            nc.sync.dma_start(out=b3_2[:], in_=b_dr[2][:].to_broadcast([2, 1]))
            gb_sb = []
            for i, (gd, bd) in enumerate(gb_dr):
                tg = consts.tile([P, 1], f32, tag=f"g{i}")
                nc.sync.dma_start(out=tg[:], in_=gd[:, None])
                tb = consts.tile([P, 1], f32, tag=f"be{i}")
                nc.sync.dma_start(out=tb[:], in_=bd[:, None])
                gb_sb.append((tg, tb))
            iota_p = consts.tile([P, P], bf16, tag="iota_p")
            nc.sync.dma_start(out=iota_p[:], in_=iota_p_d[:])
            ident = consts.tile([P, P], f32, tag="ident")
            nc.sync.dma_start(out=ident[:], in_=ident_d[:])
            eps_sb = consts.tile([P, 1], f32, tag="eps")
            nc.vector.memset(eps_sb[:], cfg.EPS)

            # ======================================================= bn tail
            def bn_scale_from_stats(layer, stat_t):
                """AR the per-core (mean, E[x^2]) and return (scal, shif)."""
                mv = small.tile([P, 2], f32, tag="mv")
                nc.vector.bn_aggr(out=mv[:], in_=stat_t[:, :, :])
                sloc = small.tile([P, 2], f32, tag="sloc")
                nc.vector.tensor_copy(out=sloc[:, 0:1], in_=mv[:, 0:1])
                nc.vector.tensor_tensor(
                    out=sloc[:, 1:2], in0=mv[:, 0:1], in1=mv[:, 0:1],
                    op=AluOp.mult,
                )
                nc.vector.tensor_add(
                    out=sloc[:, 1:2], in0=sloc[:, 1:2], in1=mv[:, 1:2]
                )
                nc.sync.dma_start(out=stat_in[layer][:], in_=sloc[:])
                nc.gpsimd.collective_compute(
                    "AllGather", AluOp.bypass,
                    replica_groups=[[0, 1, 2, 3, 4, 5, 6, 7]],
                    ins=[stat_in[layer][:]], outs=[stat_out[layer][:]],
                )
                s8 = small.tile([P, 8, 2], f32, tag="s8")
                nc.sync.dma_start(
                    out=s8[:],
                    in_=stat_out[layer][:, :].rearrange("(c p) b -> p c b",
                                                        p=P),
                )
                sglob = small.tile([P, 2], f32, tag="sglob")
                nc.vector.tensor_add(out=sglob[:], in0=s8[:, 0, :],
                                     in1=s8[:, 1, :])
                for c in range(2, 8):
                    nc.vector.tensor_add(out=sglob[:], in0=sglob[:],
                                         in1=s8[:, c, :])
                nc.scalar.mul(out=sglob[:], in_=sglob[:], mul=0.125)
                var = small.tile([P, 1], f32, tag="var")
                nc.vector.tensor_tensor(
                    out=var[:], in0=sglob[:, 0:1], in1=sglob[:, 0:1],
                    op=AluOp.mult,
                )
                nc.vector.tensor_sub(out=var[:], in0=sglob[:, 1:2], in1=var[:])
                rstd = small.tile([P, 1], f32, tag="rstd")
                nc.scalar.activation(out=rstd[:], in_=var[:], func=ActF.Sqrt,
                                     bias=eps_sb[:])
                nc.vector.reciprocal(out=rstd[:], in_=rstd[:])
                scal = small.tile([P, 1], f32, tag="scal")
                nc.vector.tensor_tensor(
                    out=scal[:], in0=gb_sb[layer][0][:], in1=rstd[:],
                    op=AluOp.mult,
                )
                shif = small.tile([P, 1], f32, tag="shif")
                nc.vector.tensor_tensor(
                    out=shif[:], in0=sglob[:, 0:1], in1=scal[:], op=AluOp.mult,
                )
                nc.vector.tensor_sub(out=shif[:], in0=gb_sb[layer][1][:],
                                     in1=shif[:])
                return scal, shif

            # ============================================ layer 0: streamed x
            hraw = [hrawp.tile([P, cfg.TILES * P], f32,
                               tag=f"hraw{b}", name=f"hraw{b}")
                    for b in range(2)]
            stat_t = statp.tile([P, 2 * cfg.TILES, 6], f32, tag="stats")

            agg_ps = None
            for ci, (c0, nch) in enumerate(calls_x):
                gt = gwp.tile([P, CPC, TW], bf16, tag="gw")
                eng = nc.sync if ci % 2 == 0 else nc.scalar
                eng.dma_start(
                    out=gt[:, :nch, :],
                    in_=xg_d[:, c0 * TW:(c0 + nch) * TW],
                )
                oh = ohp.tile([P, CPC * P], bf16, tag="oh")
                nc.vector.tensor_tensor(
                    out=oh[:, :nch * P],
                    in0=bcast_inner(wdx_sb[:, c0:c0 + nch], P),
                    in1=bcast_rep(iota_p[:], nch),
                    op=AluOp.is_equal,
                )
                for j in range(nch):
                    tt, first, last = meta_x[c0 + j]
                    if first:
                        agg_ps = [ps_agg.tile([P, P], f32, tag=f"agg{b}",
                                              name=f"agg{b}")
                                  for b in range(2)]
                    for b in range(2):
                        nc.tensor.matmul(
                            agg_ps[b][:, :],
                            lhsT=gt[:, j, b * FEAT:(b + 1) * FEAT],
                            rhs=oh[:, j * P:(j + 1) * P],
                            start=first, stop=last,
                        )
                    if last:
                        valid = cfg.VALID_LAST if tt == cfg.TILES - 1 else P
                        for b in range(2):
                            agg_sb = aggp.tile([P, P], f32, tag=f"aggsb{b}")
                            nc.scalar.activation(out=agg_sb[:],
                                                 in_=agg_ps[b][:],
                                                 func=ActF.Copy)
                            h_ps = ps_h.tile([P, P], f32, tag="hps")
                            nc.tensor.matmul(
                                h_ps[:], lhsT=w_sb[0][:],
                                rhs=agg_sb[:], start=True, stop=True,
                            )
                            nc.scalar.activation(
                                out=hraw[b][:, tt * P:tt * P + P],
                                in_=h_ps[:], func=ActF.Identity,
                                bias=b_sb[0][:],
                            )
                            nc.vector.bn_stats(
                                out=stat_t[:, 2 * tt + b, :],
                                in_=hraw[b][:, tt * P:tt * P + valid],
                            )

            # --------------------------- boundary 0: BN, pass-2, quarter AGs
            scal, shif = bn_scale_from_stats(0, stat_t)
            for b in range(2):
                nc.scalar.activation(
                    out=hraw[b][:], in_=hraw[b][:],
                    func=ActF.Relu, bias=shif[:], scale=scal[:],
                )
            for qi in range(2):
                t0, t1 = QT[2 * qi], QT[2 * qi + 2]
                for t in range(t0, t1):
                    valid = cfg.VALID_LAST if t == cfg.TILES - 1 else P
                    hrow2 = p2p.tile([P, TW], bf16, tag="hrow")
                    for b in range(2):
                        t_ps = ps_t.tile([P, P], f32, tag="tps")
                        nc.tensor.transpose(
                            out=t_ps[:], in_=hraw[b][:, t * P:(t + 1) * P],
                            identity=ident[:])
                        nc.vector.tensor_copy(
                            out=hrow2[:, b * FEAT:(b + 1) * FEAT],
                            in_=t_ps[:])
                    nc.sync.dma_start(
                        out=shard_out[t * P:t * P + valid, :],
                        in_=hrow2[:valid, :],
                    )
                r0 = QSTART[2 * qi]
                rsz = cfg.QSZ[2 * qi] + cfg.QSZ[2 * qi + 1]
                nc.gpsimd.collective_compute(
                    "AllGather", AluOp.bypass,
                    replica_groups=[[0, 1, 2, 3, 4, 5, 6, 7]],
                    ins=[shard_out[r0:r0 + rsz, :]],
                    outs=[htab[8 * r0:8 * r0 + 8 * rsz, :]],
                )

            # ====================================== layer 1: gather from htab
            hraw = [hrawp.tile([P, cfg.TILES * P], f32,
                               tag=f"hraw{b}", name=f"hraw{b}")
                    for b in range(2)]
            stat_t = statp.tile([P, 2 * cfg.TILES, 6], f32, tag="stats")
            agg_lo = [aglop.tile([P, cfg.TILES * P], bf16,
                                 tag=f"aglo{b}", name=f"aglo{b}")
                      for b in range(2)]
            qn = 0
            for (bb, c0, nch) in calls_h:
                gt = gwp.tile([P, CPC, TW], bf16, tag="gw")
                src = htab[0:cfg.HHALF, :] if bb == 0 else htab[cfg.HHALF:, :]
                nc.gpsimd.dma_gather(
                    gt[:, :nch, :], src,
                    wih_sb[:, c0 * 8:(c0 + nch) * 8],
                    nch * P, nch * P, TW,
                    queue_num=qn,
                )
                qn = (qn + 1) % 4
                oh = ohp.tile([P, CPC * P], bf16, tag="oh")
                nc.vector.tensor_tensor(
                    out=oh[:, :nch * P],
                    in0=bcast_inner(wdh_sb[:, c0:c0 + nch], P),
                    in1=bcast_rep(iota_p[:], nch),
                    op=AluOp.is_equal,
                )
                for j in range(nch):
                    tt, first, last = meta_h[c0 + j]
                    if first:
                        agg_ps = [ps_agg.tile([P, P], f32, tag=f"agg{b}",
                                              name=f"agg{b}")
                                  for b in range(2)]
                    for b in range(2):
                        nc.tensor.matmul(
                            agg_ps[b][:, :],
                            lhsT=gt[:, j, b * FEAT:(b + 1) * FEAT],
                            rhs=oh[:, j * P:(j + 1) * P],
                            start=first, stop=last,
                        )
                    if last and bb == 0:
                        for b in range(2):
                            nc.vector.tensor_copy(
                                out=agg_lo[b][:, tt * P:(tt + 1) * P],
                                in_=agg_ps[b][:],
                            )
                    elif last and bb == 1:
                        valid = cfg.VALID_LAST if tt == cfg.TILES - 1 else P
                        for b in range(2):
                            agg_sb = aggp.tile([P, P], f32, tag=f"aggsb{b}")
                            nc.vector.tensor_add(
                                out=agg_sb[:], in0=agg_ps[b][:],
                                in1=agg_lo[b][:, tt * P:(tt + 1) * P],
                            )
                            h_ps = ps_h.tile([P, P], f32, tag="hps")
                            nc.tensor.matmul(
                                h_ps[:], lhsT=w_sb[1][:],
                                rhs=agg_sb[:], start=True, stop=True,
                            )
                            nc.vector.tensor_scalar_add(
                                out=hraw[b][:, tt * P:tt * P + P],
                                in0=h_ps[:], scalar1=b_sb[1][:],
                            )
                            nc.vector.bn_stats(
                                out=stat_t[:, 2 * tt + b, :],
                                in_=hraw[b][:, tt * P:tt * P + valid],
                            )

            # ---------------- boundary 1: BN, s = relu(bn(h2)) @ W3, s AG,
            # local expansion into the 256B-row s_tab
            scal, shif = bn_scale_from_stats(1, stat_t)
            for b in range(2):
                nc.scalar.activation(
                    out=hraw[b][:], in_=hraw[b][:],
                    func=ActF.Relu, bias=shif[:], scale=scal[:],
                )
            s_sb = outp.tile([P, cfg.TILES, 2], f32, tag="s_sb")
            for t in range(cfg.TILES):
                s_ps = ps_t.tile([P, P], f32, tag="tps")
                for b in range(2):
                    nc.tensor.matmul(s_ps[:, b:b + 1],
                                     lhsT=hraw[b][:, t * P:(t + 1) * P],
                                     rhs=w_sb[2][:],
                                     start=True, stop=True)
                nc.vector.tensor_copy(out=s_sb[:, t, :], in_=s_ps[:, 0:2])
            # p-major s_shard layout: position p*49+t = local node t*128+p,
            # so the store is one contiguous 392B run per partition and the
            # host remaps gather indices to match.
            nc.sync.dma_start(
                out=s_shard[:, :].rearrange("(p t) b -> p t b", t=cfg.TILES),
                in_=s_sb[:, :, :],
            )
            nc.gpsimd.collective_compute(
                "AllGather", AluOp.bypass,
                replica_groups=[[0, 1, 2, 3, 4, 5, 6, 7]],
                ins=[s_shard[:, :]], outs=[s_full[:, :]],
            )
            # expansion: s_tab row n = [s_b0(n) x128 | s_b1(n) x128] (bf16).
            # Slab maps partition p to a CONTIGUOUS row range so the store
            # is one big contiguous run per partition (fast DMA).
            for n0 in range(0, S_PAD, SLAB_N):
                rn = min(SLAB_R, (S_PAD - n0) // P)
                sexp = sexpp.tile([P, SLAB_R * SW], bf16, tag="sexp")
                sl = sexpp.tile([P, SLAB_R * 2], f32, tag="sl")
                nc.scalar.dma_start(
                    out=sl[:, :rn * 2],
                    in_=s_full[n0:n0 + rn * P, :].rearrange(
                        "(p r) b -> p (r b)", r=rn),
                )
                for b in range(2):
                    nc.vector.tensor_copy(
                        out=re_ap(sexp[:], [[SW, rn], [1, SW // 2]],
                                  extra_off=b * (SW // 2)),
                        in_=re_ap(sl[:], [[2, rn], [0, SW // 2]],
                                  extra_off=b),
                    )
                nc.sync.dma_start(
                    out=s_tab[n0:n0 + rn * P, :].rearrange(
                        "(p r) f -> p (r f)", r=rn),
                    in_=sexp[:, :rn * SW],
                )

            # ================================= layer 2: gather scalar s rows
            out_acc = outp.tile([2, cfg.TILES * P], f32, tag="out_acc")
            qn = 0
            for (bb, c0, nch) in calls_s:
                gt = gwp.tile([P, CPC, TW], bf16, tag="gw")
                src = s_tab[0:S_HALF, :] if bb == 0 else s_tab[S_HALF:, :]
                nc.gpsimd.dma_gather(
                    gt[:, :nch, :], src,
                    wis_sb[:, c0 * 8:(c0 + nch) * 8],
                    nch * P, nch * P, SW,
                    queue_num=qn,
                )
                qn = (qn + 1) % 4
                oh = ohp.tile([P, CPC * P], bf16, tag="oh")
                nc.vector.tensor_tensor(
                    out=oh[:, :nch * P],
                    in0=bcast_inner(wds_sb[:, c0:c0 + nch], P),
                    in1=bcast_rep(iota_p[:], nch),
                    op=AluOp.is_equal,
                )
                for j in range(nch):
                    tt, first, last = meta_s[c0 + j]
                    if first:
                        agg2 = ps_agg.tile([P, P], f32, tag="agg0",
                                           name="agg0")[0:2, :]
                    nc.tensor.matmul(
                        agg2[:, :],
                        lhsT=re_ap(gt[:], [[SW // 2, 2]], extra_off=j * SW),
                        rhs=oh[:, j * P:(j + 1) * P],
                        start=first, stop=last,
                    )
                    if last and bb == 0:
                        nc.vector.tensor_copy(
                            out=out_acc[:, tt * P:(tt + 1) * P],
                            in_=agg2[:],
                        )
                    elif last and bb == 1:
                        nc.vector.tensor_add(
                            out=out_acc[:, tt * P:(tt + 1) * P],
                            in0=agg2[:],
                            in1=out_acc[:, tt * P:(tt + 1) * P],
                        )
            # bias + single output DMA
            nc.vector.tensor_scalar_add(
                out=out_acc[:, 0:cfg.SHARD], in0=out_acc[:, 0:cfg.SHARD],
                scalar1=b3_2[:],
            )
            nc.sync.dma_start(out=out_d[:, :], in_=out_acc[:, 0:cfg.SHARD])

    nc.compile()
    return nc


# ---------------------------------------------------------------- host + run


def run_gcn(cfg, inputs, trace=False):
    import ml_dtypes
    from concourse.bass_utils import run_bass_kernel_spmd

    x = np.asarray(inputs["x"], dtype=np.float32)
    edge_index = np.asarray(inputs["edge_index"])

    # half-block htab layout: [half0: cores 0-7 | half1: cores 0-7]
    qstart = np.array([0, QSTART[2], cfg.SHARD], dtype=np.int64)
    qoff = np.array([0, 8 * QSTART[2]], dtype=np.int64)
    qsz = np.array([QSTART[2], cfg.SHARD - QSTART[2]], dtype=np.int64)

    def remap_h(r):
        q, rr = r // cfg.SHARD, r % cfg.SHARD
        j = np.searchsorted(qstart, rr, side="right") - 1
        return qoff[j] + q * qsz[j] + (rr - qstart[j])

    calls_x, meta_x, src_list, wdx_list = build_schedule_x(cfg, edge_index)
    calls_h, meta_h, wih_list, wdh_list = build_schedule(cfg, edge_index,
                                                         remap_h,
                                                         half=cfg.HHALF)
    def remap_s(n):
        c, r = n // cfg.SHARD, n % cfg.SHARD
        return c * SSH + (r % P) * cfg.TILES + r // P

    calls_s, meta_s, wis_list, wds_list = build_schedule(cfg, edge_index,
                                                         remap_s,
                                                         half=S_HALF)

    # interleaved x table: row n = [x(b0,n,:) | x(b1,n,:)]; host pre-gather
    x_il = np.concatenate([x[0], x[1]], axis=1).astype(ml_dtypes.bfloat16)
    TW = 2 * FEAT
    nchx = len(meta_x)
    xg_list = []
    for q in range(8):
        g = x_il[src_list[q].reshape(-1)]  # [nchx*128, 256]
        g = g.reshape(nchx, P, TW).transpose(1, 0, 2).reshape(P, nchx * TW)
        xg_list.append(np.ascontiguousarray(g))

    shapes = {"xg": xg_list[0].shape, "wih": wih_list[0].shape,
              "wis": wis_list[0].shape, "wdx": wdx_list[0].shape,
              "wdh": wdh_list[0].shape, "wds": wds_list[0].shape}
    nc = build_nc(cfg, ((calls_x, meta_x), (calls_h, meta_h),
                        (calls_s, meta_s)), shapes)

    iota_p = np.tile(np.arange(P, dtype=np.float32), (P, 1))
    ident = np.eye(P, dtype=np.float32)
    common = {
        "W1": np.asarray(inputs["W1"], np.float32),
        "W2": np.asarray(inputs["W2"], np.float32),
        "W3": np.asarray(inputs["W3"], np.float32),
        "b1": np.asarray(inputs["b1"], np.float32),
        "b2": np.asarray(inputs["b2"], np.float32),
        "b3": np.asarray(inputs["b3"], np.float32),
        "gamma1": np.asarray(inputs["gamma1"], np.float32),
        "beta1": np.asarray(inputs["beta1"], np.float32),
        "gamma2": np.asarray(inputs["gamma2"], np.float32),
        "beta2": np.asarray(inputs["beta2"], np.float32),
        "iota_p": iota_p.astype(ml_dtypes.bfloat16),
        "ident": ident,
    }
    in_maps = []
    for c in range(8):
        m = dict(common)
        m["xg"] = xg_list[c]
        m["wih"] = wih_list[c]
        m["wis"] = wis_list[c]
        m["wdx"] = wdx_list[c].astype(ml_dtypes.bfloat16)
        m["wdh"] = wdh_list[c].astype(ml_dtypes.bfloat16)
        m["wds"] = wds_list[c].astype(ml_dtypes.bfloat16)
        in_maps.append(m)

    try:
        res = run_bass_kernel_spmd(nc, in_maps, core_ids=list(range(8)), trace=trace)
    except ModuleNotFoundError:
        res = run_bass_kernel_spmd(nc, in_maps, core_ids=list(range(8)), trace=False)
    out = np.empty((cfg.BATCH, cfg.N), np.float32)
    for c in range(8):
        out[:, c * cfg.SHARD:(c + 1) * cfg.SHARD] = res.results[c]["out"]
    return out, res


def kernel(**inputs) -> np.ndarray:
    cfg = Cfg()
    out, _ = run_gcn(cfg, inputs, trace=False)
    return out


# revision 38
# speedup vs baseline: 1.0115x; 1.0115x over previous
"""ClusterGCN (3-layer GCN, sum-aggregation) on 8 Trainium2 NeuronCores.

Strategy (hardcoded for B=2, N=50000, F=H=128, E=800000, 8 cores):
  - core c: destination shard c (6250 nodes), BOTH batches. Tables are
    batch-interleaved: row n = [h(b0,n,:) | h(b1,n,:)] in bf16 (512B), so one
    gather index fetches both batches' source rows.
  - Reassociate each layer: A @ (h @ W) == (A @ h) @ W: aggregate first
    (segment-sum over edges), then one dense 128x128 matmul per batch.
  - Edges sorted by (dst_tile, ...) into 128-slot chunks; each chunk -> one
    is_equal one-hot [slot, dst_rel] and accumulating matmuls into PSUM agg
    tiles [feat, dst].
  - LAYER 0: x is a kernel input, so the gather is STATIC -> pre-gather x on
    the HOST into chunk-slot order and stream it with plain HWDGE DMA.
    Zero SWDGE descriptors (the Q7 descriptor-gen engine is the bottleneck
    at ~4.2ns/index, ~480us per gather layer).
  - LAYER 1: SWDGE dma_gather from the block-layout htab (bf16). Gather
    calls are <=1024 idx (hard Q7 scratch limit; 2048 wedges the device),
    cycled over SWDGE queues 0..3. The htab AllGather is split into 4
    tile-range quarters so the next layer's lo-bucket gathers start after
    quarters 0+1 land, and quarters 2+3 hide behind phase-A gathers.
  - LAYER 2: output feature width is 1, so compute s = relu(bn(h2)) @ W3
    per-shard BEFORE communicating: AllGather only [50000, 2] f32 (50KB/core)
    instead of the 3.2MB/core table, then expand locally into a
    [50048, 128] bf16 table (row n = (s_b0, s_b1) replicated 64x, 256B = the
    SWDGE gather minimum elem). Layer-2 gathers fetch 256B rows and
    aggregate with [slot,2] x [slot,dst] matmuls into [2, dst] PSUM tiles.
  - BatchNorm is training-mode over all B*N rows: per-core bn_stats/bn_aggr,
    then an 8-core AllReduce of (mean, E[x^2]).
  - SPMD: one instruction stream for all 8 cores -> call schedules are
    canonical (per-group max chunk count over shards); each shard pads its
    own chunks with idx 0 / dst_rel 255 (one-hot all-zero).
"""

import math

import numpy as np

P = 128
FEAT = 128
CPC = 8  # max chunks per gather call (8*128 = 1024 idx, Q7 scratch limit)

# htab quarter-block layout (tile ranges per shard; lo bucket = q0+q1)
QT = [0, 13, 25, 37, 49]             # tile boundaries of the 4 AG pieces
QSTART = [0, 13 * P, 25 * P, 37 * P]  # row starts within a shard
SSH = 49 * P                          # padded s_shard rows per core (6272)
S_PAD = 8 * SSH                       # s_tab rows (50176)
S_HALF = 4 * SSH                      # s_tab lo/hi bucket boundary (25088)
SW = 256                              # s_tab row width in bf16 (512B)
SLAB_R = 13                           # s_tab expansion rows per partition
SLAB_N = SLAB_R * P                   # nodes per expansion slab (1664)


class Cfg:
    def __init__(self, n_nodes=50000, batch=2, eps=1e-5):
        self.N = n_nodes
        self.SHARD = n_nodes // 8  # 6250
        self.BATCH = batch
        self.HALF = n_nodes // 2
        self.TILES = math.ceil(self.SHARD / P)  # 49
        self.VALID_LAST = self.SHARD - (self.TILES - 1) * P  # 106
        self.EPS = eps
        self.QSZ = [QSTART[1], QSTART[2] - QSTART[1], QSTART[3] - QSTART[2],
                    self.SHARD - QSTART[3]]
        self.QOFF = [8 * q for q in QSTART]  # block offsets in htab
        self.HHALF = 8 * QSTART[2]  # 25600: htab lo/hi gather view split


def _wrap16(stream):
    """[n] idx stream -> [128, n/16] wrapped col-major, replicated x8."""
    return np.tile(stream.reshape(-1, 16).T, (8, 1))


def build_schedule(cfg, edge_index, remap=None, half=None):
    """Canonical dst-sorted chunk schedule shared by all 8 shards.

    remap: optional vectorized fn mapping global source ids to table rows
    (used for the quarter-block htab layout of layer 1).

    Returns (calls, chunk_meta, wi_list, wd_list):
      calls: list of (bucket, chunk0, nch) gather calls
      chunk_meta: per chunk (tile, first_in_group, last_in_group)
      wi_list[q]: [128, n_chunks*8] i16 wrapped gather idx for shard q
      wd_list[q]: [128, n_chunks] f32 per-chunk dst_rel (along partitions)
    """
    row = np.asarray(edge_index[0]).astype(np.int64)
    col = np.asarray(edge_index[1]).astype(np.int64)
    if remap is not None:
        row = remap(row)
    if half is None:
        half = cfg.HALF

    groups = []
    for q in range(8):
        base = q * cfg.SHARD
        m = (col >= base) & (col < base + cfg.SHARD)
        r = row[m]
        c = col[m] - base
        t = c // P
        drel = c % P
        bkt = (r >= half).astype(np.int64)
        order = np.lexsort((r, drel, bkt, t))
        r, t, drel, bkt = r[order], t[order], drel[order], bkt[order]
        idx16 = np.where(bkt == 1, r - half, r).astype(np.int16)
        g = {}
        key = t * 2 + bkt
        bounds = np.flatnonzero(np.append(True, key[1:] != key[:-1]))
        bounds = np.append(bounds, len(key))
        for j in range(len(bounds) - 1):
            s, e = int(bounds[j]), int(bounds[j + 1])
            g[(int(t[s]), int(bkt[s]))] = (idx16[s:e], drel[s:e].astype(np.float32))
        groups.append(g)

    kmax = {}
    for t in range(cfg.TILES):
        for b in (0, 1):
            n = max(len(g.get((t, b), ((), ()))[0]) for g in groups)
            kmax[(t, b)] = max(1, math.ceil(n / P))

    chunk_of = {}
    chunk_meta = []
    c0 = 0
    for b in (0, 1):
        for t in range(cfg.TILES):
            k = kmax[(t, b)]
            chunk_of[(t, b)] = c0
            for j in range(k):
                chunk_meta.append((t, j == 0, j == k - 1))
            c0 += k
    nch_total = c0
    calls = []
    nlo = sum(kmax[(t, 0)] for t in range(cfg.TILES))
    for b, (lo, hi) in ((0, (0, nlo)), (1, (nlo, nch_total))):
        for s2 in range(lo, hi, CPC):
            calls.append((b, s2, min(CPC, hi - s2)))

    wi_list, wd_list = [], []
    for q in range(8):
        wi = np.zeros((128, nch_total * 8), np.int16)
        wd = np.full((128, nch_total), 255.0, np.float32)
        for (t, b), cc0 in chunk_of.items():
            idx16, drel = groups[q].get((t, b), (np.zeros(0, np.int16),
                                                 np.zeros(0, np.float32)))
            k = kmax[(t, b)]
            pi = np.zeros(k * P, np.int16)
            pd = np.full(k * P, 255.0, np.float32)
            pi[:len(idx16)] = idx16
            pd[:len(drel)] = drel
            wi[:, cc0 * 8:(cc0 + k) * 8] = _wrap16(pi)
            wd[:, cc0:cc0 + k] = pd.reshape(k, P).T
        wi_list.append(wi)
        wd_list.append(wd)
    return calls, chunk_meta, wi_list, wd_list


def build_schedule_x(cfg, edge_index):
    """Single-bucket canonical schedule for layer 0 (host pre-gather).

    Returns (calls, chunk_meta, src_list, wd_list):
      calls: list of (chunk0, nch)
      src_list[q]: [nch_total, 128] int64 global source ids per slot (pad 0)
      wd_list[q]: [128, nch_total] f32 dst_rel (pad 255)
    """
    row = np.asarray(edge_index[0]).astype(np.int64)
    col = np.asarray(edge_index[1]).astype(np.int64)
    groups = []
    for q in range(8):
        base = q * cfg.SHARD
        m = (col >= base) & (col < base + cfg.SHARD)
        r = row[m]
        c = col[m] - base
        t = c // P
        drel = c % P
        order = np.lexsort((r, drel, t))
        r, t, drel = r[order], t[order], drel[order]
        g = {}
        bounds = np.flatnonzero(np.append(True, t[1:] != t[:-1]))
        bounds = np.append(bounds, len(t))
        for j in range(len(bounds) - 1):
            s, e = int(bounds[j]), int(bounds[j + 1])
            g[int(t[s])] = (r[s:e], drel[s:e].astype(np.float32))
        groups.append(g)

    kmax = []
    for t in range(cfg.TILES):
        n = max(len(g.get(t, ((), ()))[0]) for g in groups)
        kmax.append(max(1, math.ceil(n / P)))
    chunk_meta = []
    for t in range(cfg.TILES):
        for j in range(kmax[t]):
            chunk_meta.append((t, j == 0, j == kmax[t] - 1))
    nch_total = len(chunk_meta)
    calls = [(s2, min(CPC, nch_total - s2)) for s2 in range(0, nch_total, CPC)]

    src_list, wd_list = [], []
    for q in range(8):
        srcs = np.zeros((nch_total, P), np.int64)
        wd = np.full((128, nch_total), 255.0, np.float32)
        c0 = 0
        for t in range(cfg.TILES):
            r, drel = groups[q].get(t, (np.zeros(0, np.int64),
                                        np.zeros(0, np.float32)))
            k = kmax[t]
            pr = np.zeros(k * P, np.int64)
            pd = np.full(k * P, 255.0, np.float32)
            pr[:len(r)] = r
            pd[:len(drel)] = drel
            srcs[c0:c0 + k] = pr.reshape(k, P)
            wd[:, c0:c0 + k] = pd.reshape(k, P).T
            c0 += k
        src_list.append(srcs)
        wd_list.append(wd)
    return calls, chunk_meta, src_list, wd_list


# ---------------------------------------------------------------- bass kernel


def build_nc(cfg, scheds, shapes):
    import concourse.bacc as bacc
    import concourse.bass as bass
    import concourse.tile as tile
    from concourse import mybir

    f32 = mybir.dt.float32
    bf16 = mybir.dt.bfloat16
    i16 = mybir.dt.int16
    TW = 2 * FEAT  # interleaved table width (256)

    (calls_x, meta_x), (calls_h, meta_h), (calls_s, meta_s) = scheds

    nc = bacc.Bacc("TRN2", target_bir_lowering=False, debug=False,
                   num_devices=8, num_swdge_queues=4)

    xg_d = nc.dram_tensor("xg", list(shapes["xg"]), bf16, kind="ExternalInput")
    wih_d = nc.dram_tensor("wih", list(shapes["wih"]), i16, kind="ExternalInput")
    wis_d = nc.dram_tensor("wis", list(shapes["wis"]), i16, kind="ExternalInput")
    wdx_d = nc.dram_tensor("wdx", list(shapes["wdx"]), bf16, kind="ExternalInput")
    wdh_d = nc.dram_tensor("wdh", list(shapes["wdh"]), bf16, kind="ExternalInput")
    wds_d = nc.dram_tensor("wds", list(shapes["wds"]), bf16, kind="ExternalInput")
    w_dr = [nc.dram_tensor(f"W{i+1}", [FEAT, FEAT if i < 2 else 1], f32,
                           kind="ExternalInput") for i in range(3)]
    b_dr = [nc.dram_tensor(f"b{i+1}", [FEAT if i < 2 else 1], f32,
                           kind="ExternalInput") for i in range(3)]
    gb_dr = [(nc.dram_tensor(f"gamma{i+1}", [FEAT], f32, kind="ExternalInput"),
              nc.dram_tensor(f"beta{i+1}", [FEAT], f32, kind="ExternalInput"))
             for i in range(2)]
    iota_p_d = nc.dram_tensor("iota_p", [P, P], bf16, kind="ExternalInput")
    ident_d = nc.dram_tensor("ident", [P, P], f32, kind="ExternalInput")
    out_d = nc.dram_tensor("out", [cfg.BATCH, cfg.SHARD], f32, kind="ExternalOutput")

    htab = nc.dram_tensor("htab0", [cfg.N, TW], bf16, kind="Internal",
                          addr_space="Shared")
    shard_out = nc.dram_tensor("shard_out0", [cfg.SHARD, TW], bf16,
                               kind="Internal")
    s_shard = nc.dram_tensor("s_shard", [SSH, 2], f32, kind="Internal")
    s_full = nc.dram_tensor("s_full", [S_PAD, 2], f32, kind="Internal")
    s_tab = nc.dram_tensor("s_tab", [S_PAD, SW], bf16, kind="Internal")
    stat_in = [nc.dram_tensor(f"stat_in{i}", [P, 2], f32, kind="Internal")
               for i in range(2)]
    stat_out = [nc.dram_tensor(f"stat_out{i}", [8 * P, 2], f32,
                               kind="Internal", addr_space="Shared")
                for i in range(2)]

    AluOp = mybir.AluOpType
    ActF = mybir.ActivationFunctionType

    def bcast_inner(ap, inner):
        return bass.AP(tensor=ap.tensor, offset=ap.offset,
                       ap=[list(ap.ap[0]), list(ap.ap[1]), [0, inner]])

    def bcast_rep(ap, reps):
        return bass.AP(tensor=ap.tensor, offset=ap.offset,
                       ap=[list(ap.ap[0]), [0, reps], list(ap.ap[1])])

    def re_ap(ap, free_dims, extra_off=0):
        """Same tensor/partition dim, custom free-dim APs."""
        return bass.AP(tensor=ap.tensor, offset=ap.offset + extra_off,
                       ap=[list(ap.ap[0])] + [list(d) for d in free_dims])

    with tile.TileContext(nc) as tc:
        with (
            tc.tile_pool(name="consts", bufs=1) as consts,
            tc.tile_pool(name="gw", bufs=9) as gwp,
            tc.tile_pool(name="ohp", bufs=6) as ohp,
            tc.tile_pool(name="aggp", bufs=2) as aggp,
            tc.tile_pool(name="hraw", bufs=1) as hrawp,
            tc.tile_pool(name="aglo", bufs=1) as aglop,
            tc.tile_pool(name="statp", bufs=1) as statp,
            tc.tile_pool(name="small", bufs=8) as small,
            tc.tile_pool(name="p2", bufs=6) as p2p,
            tc.tile_pool(name="sexp", bufs=2) as sexpp,
            tc.tile_pool(name="outp", bufs=1) as outp,
            tc.tile_pool(name="ps_agg", bufs=2, space="PSUM") as ps_agg,
            tc.tile_pool(name="ps_h", bufs=2, space="PSUM") as ps_h,
            tc.tile_pool(name="ps_t", bufs=2, space="PSUM") as ps_t,
        ):
            wih_sb = consts.tile(list(shapes["wih"]), i16, tag="wih")
            nc.sync.dma_start(out=wih_sb[:], in_=wih_d[:])
            wis_sb = consts.tile(list(shapes["wis"]), i16, tag="wis")
            nc.sync.dma_start(out=wis_sb[:], in_=wis_d[:])
            wdx_sb = consts.tile(list(shapes["wdx"]), bf16, tag="wdx")
            nc.sync.dma_start(out=wdx_sb[:], in_=wdx_d[:])
            wdh_sb = consts.tile(list(shapes["wdh"]), bf16, tag="wdh")
            nc.sync.dma_start(out=wdh_sb[:], in_=wdh_d[:])
            wds_sb = consts.tile(list(shapes["wds"]), bf16, tag="wds")
            nc.sync.dma_start(out=wds_sb[:], in_=wds_d[:])
            w_sb = []
            for i, wdr in enumerate(w_dr):
                t = consts.tile([P, FEAT if i < 2 else 1], f32, tag=f"w{i}")
                nc.sync.dma_start(out=t[:], in_=wdr[:])
                w_sb.append(t)
            b_sb = []
            for i in range(2):
                t = consts.tile([P, 1], f32, tag=f"b{i}")
                nc.sync.dma_start(out=t[:], in_=b_dr[i][:, None])
                b_sb.append(t)
            b3_2 = consts.tile([2, 1], f32, tag="b3_2")
            nc.sync.dma_start(out=b3_2[:], in_=b_dr[2][:].to_broadcast([2, 1]))
            gb_sb = []
            for i, (gd, bd) in enumerate(gb_dr):
                tg = consts.tile([P, 1], f32, tag=f"g{i}")
                nc.sync.dma_start(out=tg[:], in_=gd[:, None])
                tb = consts.tile([P, 1], f32, tag=f"be{i}")
                nc.sync.dma_start(out=tb[:], in_=bd[:, None])
                gb_sb.append((tg, tb))
            iota_p = consts.tile([P, P], bf16, tag="iota_p")
            nc.sync.dma_start(out=iota_p[:], in_=iota_p_d[:])
            ident = consts.tile([P, P], f32, tag="ident")
            nc.sync.dma_start(out=ident[:], in_=ident_d[:])
            eps_sb = consts.tile([P, 1], f32, tag="eps")
            nc.vector.memset(eps_sb[:], cfg.EPS)

            # ======================================================= bn tail
            def bn_scale_from_stats(layer, stat_t):
                """AR the per-core (mean, E[x^2]) and return (scal, shif)."""
                mv = small.tile([P, 2], f32, tag="mv")
                nc.vector.bn_aggr(out=mv[:], in_=stat_t[:, :, :])
                sloc = small.tile([P, 2], f32, tag="sloc")
                nc.vector.tensor_copy(out=sloc[:, 0:1], in_=mv[:, 0:1])
                nc.vector.tensor_tensor(
                    out=sloc[:, 1:2], in0=mv[:, 0:1], in1=mv[:, 0:1],
                    op=AluOp.mult,
                )
                nc.vector.tensor_add(
                    out=sloc[:, 1:2], in0=sloc[:, 1:2], in1=mv[:, 1:2]
                )
                nc.sync.dma_start(out=stat_in[layer][:], in_=sloc[:])
                nc.gpsimd.collective_compute(
                    "AllGather", AluOp.bypass,
                    replica_groups=[[0, 1, 2, 3, 4, 5, 6, 7]],
                    ins=[stat_in[layer][:]], outs=[stat_out[layer][:]],
                )
                s8 = small.tile([P, 8, 2], f32, tag="s8")
                nc.sync.dma_start(
                    out=s8[:],
                    in_=stat_out[layer][:, :].rearrange("(c p) b -> p c b",
                                                        p=P),
                )
                sglob = small.tile([P, 2], f32, tag="sglob")
                nc.vector.tensor_add(out=sglob[:], in0=s8[:, 0, :],
                                     in1=s8[:, 1, :])
                for c in range(2, 8):
                    nc.vector.tensor_add(out=sglob[:], in0=sglob[:],
                                         in1=s8[:, c, :])
                nc.scalar.mul(out=sglob[:], in_=sglob[:], mul=0.125)
                var = small.tile([P, 1], f32, tag="var")
                nc.vector.tensor_tensor(
                    out=var[:], in0=sglob[:, 0:1], in1=sglob[:, 0:1],
                    op=AluOp.mult,
                )
                nc.vector.tensor_sub(out=var[:], in0=sglob[:, 1:2], in1=var[:])
                rstd = small.tile([P, 1], f32, tag="rstd")
                nc.scalar.activation(out=rstd[:], in_=var[:], func=ActF.Sqrt,
                                     bias=eps_sb[:])
                nc.vector.reciprocal(out=rstd[:], in_=rstd[:])
                scal = small.tile([P, 1], f32, tag="scal")
                nc.vector.tensor_tensor(
                    out=scal[:], in0=gb_sb[layer][0][:], in1=rstd[:],
                    op=AluOp.mult,
                )
                shif = small.tile([P, 1], f32, tag="shif")
                nc.vector.tensor_tensor(
                    out=shif[:], in0=sglob[:, 0:1], in1=scal[:], op=AluOp.mult,
                )
                nc.vector.tensor_sub(out=shif[:], in0=gb_sb[layer][1][:],
                                     in1=shif[:])
                return scal, shif

            # ============================================ layer 0: streamed x
            hraw = [hrawp.tile([P, cfg.TILES * P], f32,
                               tag=f"hraw{b}", name=f"hraw{b}")
                    for b in range(2)]
            stat_t = statp.tile([P, 2 * cfg.TILES, 6], f32, tag="stats")

            agg_ps = None
            for ci, (c0, nch) in enumerate(calls_x):
                gt = gwp.tile([P, CPC, TW], bf16, tag="gw")
                eng = nc.sync if ci % 2 == 0 else nc.scalar
                eng.dma_start(
                    out=gt[:, :nch, :],
                    in_=xg_d[:, c0 * TW:(c0 + nch) * TW],
                )
                oh = ohp.tile([P, CPC * P], bf16, tag="oh")
                nc.vector.tensor_tensor(
                    out=oh[:, :nch * P],
                    in0=bcast_inner(wdx_sb[:, c0:c0 + nch], P),
                    in1=bcast_rep(iota_p[:], nch),
                    op=AluOp.is_equal,
                )
                for j in range(nch):
                    tt, first, last = meta_x[c0 + j]
                    if first:
                        agg_ps = [ps_agg.tile([P, P], f32, tag=f"agg{b}",
                                              name=f"agg{b}")
                                  for b in range(2)]
                    for b in range(2):
                        nc.tensor.matmul(
                            agg_ps[b][:, :],
                            lhsT=gt[:, j, b * FEAT:(b + 1) * FEAT],
                            rhs=oh[:, j * P:(j + 1) * P],
                            start=first, stop=last,
                        )
                    if last:
                        valid = cfg.VALID_LAST if tt == cfg.TILES - 1 else P
                        for b in range(2):
                            agg_sb = aggp.tile([P, P], f32, tag=f"aggsb{b}")
                            nc.scalar.activation(out=agg_sb[:],
                                                 in_=agg_ps[b][:],
                                                 func=ActF.Copy)
                            h_ps = ps_h.tile([P, P], f32, tag="hps")
                            nc.tensor.matmul(
                                h_ps[:], lhsT=w_sb[0][:],
                                rhs=agg_sb[:], start=True, stop=True,
                            )
                            nc.scalar.activation(
                                out=hraw[b][:, tt * P:tt * P + P],
                                in_=h_ps[:], func=ActF.Identity,
                                bias=b_sb[0][:],
                            )
                            nc.vector.bn_stats(
                                out=stat_t[:, 2 * tt + b, :],
                                in_=hraw[b][:, tt * P:tt * P + valid],
                            )

            # --------------------------- boundary 0: BN, pass-2, quarter AGs
            scal, shif = bn_scale_from_stats(0, stat_t)
            for b in range(2):
                nc.scalar.activation(
                    out=hraw[b][:], in_=hraw[b][:],
                    func=ActF.Relu, bias=shif[:], scale=scal[:],
                )
            for qi in range(2):
                t0, t1 = QT[2 * qi], QT[2 * qi + 2]
                for t in range(t0, t1):
                    valid = cfg.VALID_LAST if t == cfg.TILES - 1 else P
                    hrow2 = p2p.tile([P, TW], bf16, tag="hrow")
                    for b in range(2):
                        t_ps = ps_t.tile([P, P], f32, tag="tps")
                        nc.tensor.transpose(
                            out=t_ps[:], in_=hraw[b][:, t * P:(t + 1) * P],
                            identity=ident[:])
                        nc.vector.tensor_copy(
                            out=hrow2[:, b * FEAT:(b + 1) * FEAT],
                            in_=t_ps[:])
                    nc.sync.dma_start(
                        out=shard_out[t * P:t * P + valid, :],
                        in_=hrow2[:valid, :],
                    )
                r0 = QSTART[2 * qi]
                rsz = cfg.QSZ[2 * qi] + cfg.QSZ[2 * qi + 1]
                nc.gpsimd.collective_compute(
                    "AllGather", AluOp.bypass,
                    replica_groups=[[0, 1, 2, 3, 4, 5, 6, 7]],
                    ins=[shard_out[r0:r0 + rsz, :]],
                    outs=[htab[8 * r0:8 * r0 + 8 * rsz, :]],
                )

            # ====================================== layer 1: gather from htab
            hraw = [hrawp.tile([P, cfg.TILES * P], f32,
                               tag=f"hraw{b}", name=f"hraw{b}")
                    for b in range(2)]
            stat_t = statp.tile([P, 2 * cfg.TILES, 6], f32, tag="stats")
            agg_lo = [aglop.tile([P, cfg.TILES * P], bf16,
                                 tag=f"aglo{b}", name=f"aglo{b}")
                      for b in range(2)]
            qn = 0
            for (bb, c0, nch) in calls_h:
                gt = gwp.tile([P, CPC, TW], bf16, tag="gw")
                src = htab[0:cfg.HHALF, :] if bb == 0 else htab[cfg.HHALF:, :]
                nc.gpsimd.dma_gather(
                    gt[:, :nch, :], src,
                    wih_sb[:, c0 * 8:(c0 + nch) * 8],
                    nch * P, nch * P, TW,
                    queue_num=qn,
                )
                qn = (qn + 1) % 4
                oh = ohp.tile([P, CPC * P], bf16, tag="oh")
                nc.vector.tensor_tensor(
                    out=oh[:, :nch * P],
                    in0=bcast_inner(wdh_sb[:, c0:c0 + nch], P),
                    in1=bcast_rep(iota_p[:], nch),
                    op=AluOp.is_equal,
                )
                for j in range(nch):
                    tt, first, last = meta_h[c0 + j]
                    if first:
                        agg_ps = [ps_agg.tile([P, P], f32, tag=f"agg{b}",
                                              name=f"agg{b}")
                                  for b in range(2)]
                    for b in range(2):
                        nc.tensor.matmul(
                            agg_ps[b][:, :],
                            lhsT=gt[:, j, b * FEAT:(b + 1) * FEAT],
                            rhs=oh[:, j * P:(j + 1) * P],
                            start=first, stop=last,
                        )
                    if last and bb == 0:
                        for b in range(2):
                            nc.vector.tensor_copy(
                                out=agg_lo[b][:, tt * P:(tt + 1) * P],
                                in_=agg_ps[b][:],
                            )
                    elif last and bb == 1:
                        valid = cfg.VALID_LAST if tt == cfg.TILES - 1 else P
                        for b in range(2):
                            agg_sb = aggp.tile([P, P], f32, tag=f"aggsb{b}")
                            nc.vector.tensor_add(
                                out=agg_sb[:], in0=agg_ps[b][:],
                                in1=agg_lo[b][:, tt * P:(tt + 1) * P],
                            )
                            h_ps = ps_h.tile([P, P], f32, tag="hps")
                            nc.tensor.matmul(
                                h_ps[:], lhsT=w_sb[1][:],
                                rhs=agg_sb[:], start=True, stop=True,
                            )
                            nc.vector.tensor_scalar_add(
                                out=hraw[b][:, tt * P:tt * P + P],
                                in0=h_ps[:], scalar1=b_sb[1][:],
                            )
                            nc.vector.bn_stats(
                                out=stat_t[:, 2 * tt + b, :],
                                in_=hraw[b][:, tt * P:tt * P + valid],
                            )

            # ---------------- boundary 1: BN, s = relu(bn(h2)) @ W3, s AG,
            # local expansion into the 256B-row s_tab
            scal, shif = bn_scale_from_stats(1, stat_t)
            for b in range(2):
                nc.scalar.activation(
                    out=hraw[b][:], in_=hraw[b][:],
                    func=ActF.Relu, bias=shif[:], scale=scal[:],
                )
            s_sb = outp.tile([P, cfg.TILES, 2], f32, tag="s_sb")
            for t in range(cfg.TILES):
                s_ps = ps_t.tile([P, P], f32, tag="tps")
                for b in range(2):
                    nc.tensor.matmul(s_ps[:, b:b + 1],
                                     lhsT=hraw[b][:, t * P:(t + 1) * P],
                                     rhs=w_sb[2][:],
                                     start=True, stop=True)
                nc.vector.tensor_copy(out=s_sb[:, t, :], in_=s_ps[:, 0:2])
            # p-major s_shard layout: position p*49+t = local node t*128+p,
            # so the store is one contiguous 392B run per partition and the
            # host remaps gather indices to match.
            nc.sync.dma_start(
                out=s_shard[:, :].rearrange("(p t) b -> p t b", t=cfg.TILES),
                in_=s_sb[:, :, :],
            )
            nc.gpsimd.collective_compute(
                "AllGather", AluOp.bypass,
                replica_groups=[[0, 1, 2, 3, 4, 5, 6, 7]],
                ins=[s_shard[:, :]], outs=[s_full[:, :]],
            )
            # expansion: s_tab row n = [s_b0(n) x128 | s_b1(n) x128] (bf16).
            # Slab maps partition p to a CONTIGUOUS row range so the store
            # is one big contiguous run per partition (fast DMA).
            for n0 in range(0, S_PAD, SLAB_N):
                rn = min(SLAB_R, (S_PAD - n0) // P)
                sexp = sexpp.tile([P, SLAB_R * SW], bf16, tag="sexp")
                sl = sexpp.tile([P, SLAB_R * 2], f32, tag="sl")
                nc.scalar.dma_start(
                    out=sl[:, :rn * 2],
                    in_=s_full[n0:n0 + rn * P, :].rearrange(
                        "(p r) b -> p (r b)", r=rn),
                )
                for b in range(2):
                    nc.vector.tensor_copy(
                        out=re_ap(sexp[:], [[SW, rn], [1, SW // 2]],
                                  extra_off=b * (SW // 2)),
                        in_=re_ap(sl[:], [[2, rn], [0, SW // 2]],
                                  extra_off=b),
                    )
                nc.sync.dma_start(
                    out=s_tab[n0:n0 + rn * P, :].rearrange(
                        "(p r) f -> p (r f)", r=rn),
                    in_=sexp[:, :rn * SW],
                )

            # ================================= layer 2: gather scalar s rows
            out_acc = outp.tile([2, cfg.TILES * P], f32, tag="out_acc")
            qn = 0
            for (bb, c0, nch) in calls_s:
                gt = gwp.tile([P, CPC, TW], bf16, tag="gw")
                src = s_tab[0:S_HALF, :] if bb == 0 else s_tab[S_HALF:, :]
                nc.gpsimd.dma_gather(
                    gt[:, :nch, :], src,
                    wis_sb[:, c0 * 8:(c0 + nch) * 8],
                    nch * P, nch * P, SW,
                    queue_num=qn,
                )
                qn = (qn + 1) % 4
                oh = ohp.tile([P, CPC * P], bf16, tag="oh")
                nc.vector.tensor_tensor(
                    out=oh[:, :nch * P],
                    in0=bcast_inner(wds_sb[:, c0:c0 + nch], P),
                    in1=bcast_rep(iota_p[:], nch),
                    op=AluOp.is_equal,
                )
                for j in range(nch):
                    tt, first, last = meta_s[c0 + j]
                    if first:
                        agg2 = ps_agg.tile([P, P], f32, tag="agg0",
                                           name="agg0")[0:2, :]
                    nc.tensor.matmul(
                        agg2[:, :],
                        lhsT=re_ap(gt[:], [[SW // 2, 2]], extra_off=j * SW),
                        rhs=oh[:, j * P:(j + 1) * P],
                        start=first, stop=last,
                    )
                    if last and bb == 0:
                        nc.vector.tensor_copy(
                            out=out_acc[:, tt * P:(tt + 1) * P],
                            in_=agg2[:],
                        )
                    elif last and bb == 1:
                        nc.vector.tensor_add(
                            out=out_acc[:, tt * P:(tt + 1) * P],
                            in0=agg2[:],
                            in1=out_acc[:, tt * P:(tt + 1) * P],
                        )
            # bias + single output DMA
            nc.vector.tensor_scalar_add(
                out=out_acc[:, 0:cfg.SHARD], in0=out_acc[:, 0:cfg.SHARD],
                scalar1=b3_2[:],
            )
            nc.sync.dma_start(out=out_d[:, :], in_=out_acc[:, 0:cfg.SHARD])

    nc.compile()
    return nc


# ---------------------------------------------------------------- host + run


def run_gcn(cfg, inputs, trace=False):
    import ml_dtypes
    from concourse.bass_utils import run_bass_kernel_spmd

    x = np.asarray(inputs["x"], dtype=np.float32)
    edge_index = np.asarray(inputs["edge_index"])

    # half-block htab layout: [half0: cores 0-7 | half1: cores 0-7]
    qstart = np.array([0, QSTART[2], cfg.SHARD], dtype=np.int64)
    qoff = np.array([0, 8 * QSTART[2]], dtype=np.int64)
    qsz = np.array([QSTART[2], cfg.SHARD - QSTART[2]], dtype=np.int64)

    def remap_h(r):
        q, rr = r // cfg.SHARD, r % cfg.SHARD
        j = np.searchsorted(qstart, rr, side="right") - 1
        return qoff[j] + q * qsz[j] + (rr - qstart[j])

    calls_x, meta_x, src_list, wdx_list = build_schedule_x(cfg, edge_index)
    calls_h, meta_h, wih_list, wdh_list = build_schedule(cfg, edge_index,
                                                         remap_h,
                                                         half=cfg.HHALF)
    def remap_s(n):
        c, r = n // cfg.SHARD, n % cfg.SHARD
        return c * SSH + (r % P) * cfg.TILES + r // P

    calls_s, meta_s, wis_list, wds_list = build_schedule(cfg, edge_index,
                                                         remap_s,
                                                         half=S_HALF)

    # interleaved x table: row n = [x(b0,n,:) | x(b1,n,:)]; host pre-gather
    x_il = np.concatenate([x[0], x[1]], axis=1).astype(ml_dtypes.bfloat16)
    TW = 2 * FEAT
    nchx = len(meta_x)
    xg_list = []
    for q in range(8):
        g = x_il[src_list[q].reshape(-1)]  # [nchx*128, 256]
        g = g.reshape(nchx, P, TW).transpose(1, 0, 2).reshape(P, nchx * TW)
        xg_list.append(np.ascontiguousarray(g))

    shapes = {"xg": xg_list[0].shape, "wih": wih_list[0].shape,
              "wis": wis_list[0].shape, "wdx": wdx_list[0].shape,
              "wdh": wdh_list[0].shape, "wds": wds_list[0].shape}
    nc = build_nc(cfg, ((calls_x, meta_x), (calls_h, meta_h),
                        (calls_s, meta_s)), shapes)

    iota_p = np.tile(np.arange(P, dtype=np.float32), (P, 1))
    ident = np.eye(P, dtype=np.float32)
    common = {
        "W1": np.asarray(inputs["W1"], np.float32),
        "W2": np.asarray(inputs["W2"], np.float32),
        "W3": np.asarray(inputs["W3"], np.float32),
        "b1": np.asarray(inputs["b1"], np.float32),
        "b2": np.asarray(inputs["b2"], np.float32),
        "b3": np.asarray(inputs["b3"], np.float32),
        "gamma1": np.asarray(inputs["gamma1"], np.float32),
        "beta1": np.asarray(inputs["beta1"], np.float32),
        "gamma2": np.asarray(inputs["gamma2"], np.float32),
        "beta2": np.asarray(inputs["beta2"], np.float32),
        "iota_p": iota_p.astype(ml_dtypes.bfloat16),
        "ident": ident,
    }
    in_maps = []
    for c in range(8):
        m = dict(common)
        m["xg"] = xg_list[c]
        m["wih"] = wih_list[c]
        m["wis"] = wis_list[c]
        m["wdx"] = wdx_list[c].astype(ml_dtypes.bfloat16)
        m["wdh"] = wdh_list[c].astype(ml_dtypes.bfloat16)
        m["wds"] = wds_list[c].astype(ml_dtypes.bfloat16)
        in_maps.append(m)

    try:
        res = run_bass_kernel_spmd(nc, in_maps, core_ids=list(range(8)), trace=trace)
    except ModuleNotFoundError:
        res = run_bass_kernel_spmd(nc, in_maps, core_ids=list(range(8)), trace=False)
    out = np.empty((cfg.BATCH, cfg.N), np.float32)
    for c in range(8):
        out[:, c * cfg.SHARD:(c + 1) * cfg.SHARD] = res.results[c]["out"]
    return out, res


def kernel(**inputs) -> np.ndarray:
    cfg = Cfg()
    out, _ = run_gcn(cfg, inputs, trace=False)
    return out


# revision 40
# speedup vs baseline: 1.0491x; 1.0372x over previous
"""ClusterGCN (3-layer GCN, sum-aggregation) on 8 Trainium2 NeuronCores.

Strategy (hardcoded for B=2, N=50000, F=H=128, E=800000, 8 cores):
  - core c: destination shard c (6250 nodes), BOTH batches. Tables are
    batch-interleaved: row n = [h(b0,n,:) | h(b1,n,:)] in bf16 (512B), so one
    gather index fetches both batches' source rows.
  - Reassociate each layer: A @ (h @ W) == (A @ h) @ W: aggregate first
    (segment-sum over edges), then one dense 128x128 matmul per batch.
  - Edges sorted by (dst_tile, ...) into 128-slot chunks; each chunk -> one
    is_equal one-hot [slot, dst_rel] and accumulating matmuls into PSUM agg
    tiles [feat, dst].
  - LAYER 0: x is a kernel input, so the gather is STATIC -> pre-gather x on
    the HOST into chunk-slot order and stream it with plain HWDGE DMA.
    Zero SWDGE descriptors (the Q7 descriptor-gen engine is the bottleneck
    at ~4.2ns/index, ~480us per gather layer).
  - LAYER 1: SWDGE dma_gather from the half-block htab (bf16, Shared addr
    space for the fast AllGather path). Gather calls are <=1024 idx (hard
    Q7 scratch limit; 2048 wedges the device), cycled over SWDGE queues
    0..3 (the 4 Q7 core pairs), gw pool 9 bufs deep: the stream runs at
    ~4.1us/call, bounded jointly by Q7 desc-gen (~3.5us/call) and the SDMA
    drain of 1024 random 512B reads. Sub-512B gather elems hit SBUF
    read-modify-write on the S2M side and drain ~50% slower - keep 512B.
  - LAYER 2: output feature width is 1, so compute s = relu(bn(h2)) @ W3
    per-shard BEFORE communicating: AllGather only [6272, 2] f32 (50KB/core,
    p-major padded order for a contiguous store) instead of the 3.2MB/core
    table, then expand locally into a [50176, 256] bf16 table (row n =
    [s_b0 x128 | s_b1 x128], 512B rows; partition p of an expansion slab
    owns a contiguous row range so the store is one run per partition).
    Layer-2 gathers fetch 512B rows and aggregate with [slot,2] x
    [slot,dst] matmuls into [2, dst] PSUM tiles; one DMA writes the output.
  - BatchNorm is training-mode over all B*N rows: per-core bn_stats/bn_aggr,
    then an 8-core AllGather of (mean, E[x^2]) + local sum (cheaper than
    AllReduce). PSUM->SBUF copies and bias adds ride the Scalar (ACT)
    engine to unload the DVE.
  - SPMD: one instruction stream for all 8 cores -> call schedules are
    canonical (per-group max chunk count over shards); each shard pads its
    own chunks with idx 0 / dst_rel 255 (one-hot all-zero).
"""

import math

import numpy as np

P = 128
FEAT = 128
CPC = 8  # max chunks per gather call (8*128 = 1024 idx, Q7 scratch limit)

# htab quarter-block layout (tile ranges per shard; lo bucket = q0+q1)
QT = [0, 13, 25, 37, 49]             # tile boundaries of the 4 AG pieces
QSTART = [0, 13 * P, 25 * P, 37 * P]  # row starts within a shard
SSH = 49 * P                          # padded s_shard rows per core (6272)
S_PAD = 8 * SSH                       # s_tab rows (50176)
S_HALF = 4 * SSH                      # s_tab lo/hi bucket boundary (25088)
SW = 256                              # s_tab row width in bf16 (512B)
SLAB_R = 13                           # s_tab expansion rows per partition
SLAB_N = SLAB_R * P                   # nodes per expansion slab (1664)


class Cfg:
    def __init__(self, n_nodes=50000, batch=2, eps=1e-5):
        self.N = n_nodes
        self.SHARD = n_nodes // 8  # 6250
        self.BATCH = batch
        self.HALF = n_nodes // 2
        self.TILES = math.ceil(self.SHARD / P)  # 49
        self.VALID_LAST = self.SHARD - (self.TILES - 1) * P  # 106
        self.EPS = eps
        self.QSZ = [QSTART[1], QSTART[2] - QSTART[1], QSTART[3] - QSTART[2],
                    self.SHARD - QSTART[3]]
        self.QOFF = [8 * q for q in QSTART]  # block offsets in htab
        self.HHALF = 8 * QSTART[2]  # 25600: htab lo/hi gather view split


def _wrap16(stream):
    """[n] idx stream -> [128, n/16] wrapped col-major, replicated x8."""
    return np.tile(stream.reshape(-1, 16).T, (8, 1))


def build_schedule(cfg, edge_index, remap=None, half=None):
    """Canonical dst-sorted chunk schedule shared by all 8 shards.

    remap: optional vectorized fn mapping global source ids to table rows
    (used for the quarter-block htab layout of layer 1).

    Returns (calls, chunk_meta, wi_list, wd_list):
      calls: list of (bucket, chunk0, nch) gather calls
      chunk_meta: per chunk (tile, first_in_group, last_in_group)
      wi_list[q]: [128, n_chunks*8] i16 wrapped gather idx for shard q
      wd_list[q]: [128, n_chunks] f32 per-chunk dst_rel (along partitions)
    """
    row = np.asarray(edge_index[0]).astype(np.int64)
    col = np.asarray(edge_index[1]).astype(np.int64)
    if remap is not None:
        row = remap(row)
    if half is None:
        half = cfg.HALF

    groups = []
    for q in range(8):
        base = q * cfg.SHARD
        m = (col >= base) & (col < base + cfg.SHARD)
        r = row[m]
        c = col[m] - base
        t = c // P
        drel = c % P
        bkt = (r >= half).astype(np.int64)
        order = np.lexsort((r, drel, bkt, t))
        r, t, drel, bkt = r[order], t[order], drel[order], bkt[order]
        idx16 = np.where(bkt == 1, r - half, r).astype(np.int16)
        g = {}
        key = t * 2 + bkt
        bounds = np.flatnonzero(np.append(True, key[1:] != key[:-1]))
        bounds = np.append(bounds, len(key))
        for j in range(len(bounds) - 1):
            s, e = int(bounds[j]), int(bounds[j + 1])
            g[(int(t[s]), int(bkt[s]))] = (idx16[s:e], drel[s:e].astype(np.float32))
        groups.append(g)

    kmax = {}
    for t in range(cfg.TILES):
        for b in (0, 1):
            n = max(len(g.get((t, b), ((), ()))[0]) for g in groups)
            kmax[(t, b)] = max(1, math.ceil(n / P))

    chunk_of = {}
    chunk_meta = []
    c0 = 0
    for b in (0, 1):
        for t in range(cfg.TILES):
            k = kmax[(t, b)]
            chunk_of[(t, b)] = c0
            for j in range(k):
                chunk_meta.append((t, j == 0, j == k - 1))
            c0 += k
    nch_total = c0
    calls = []
    nlo = sum(kmax[(t, 0)] for t in range(cfg.TILES))
    for b, (lo, hi) in ((0, (0, nlo)), (1, (nlo, nch_total))):
        for s2 in range(lo, hi, CPC):
            calls.append((b, s2, min(CPC, hi - s2)))

    wi_list, wd_list = [], []
    for q in range(8):
        wi = np.zeros((128, nch_total * 8), np.int16)
        wd = np.full((128, nch_total), 255.0, np.float32)
        for (t, b), cc0 in chunk_of.items():
            idx16, drel = groups[q].get((t, b), (np.zeros(0, np.int16),
                                                 np.zeros(0, np.float32)))
            k = kmax[(t, b)]
            pi = np.zeros(k * P, np.int16)
            pd = np.full(k * P, 255.0, np.float32)
            pi[:len(idx16)] = idx16
            pd[:len(drel)] = drel
            wi[:, cc0 * 8:(cc0 + k) * 8] = _wrap16(pi)
            wd[:, cc0:cc0 + k] = pd.reshape(k, P).T
        wi_list.append(wi)
        wd_list.append(wd)
    return calls, chunk_meta, wi_list, wd_list


def build_schedule_x(cfg, edge_index):
    """Single-bucket canonical schedule for layer 0 (host pre-gather).

    Returns (calls, chunk_meta, src_list, wd_list):
      calls: list of (chunk0, nch)
      src_list[q]: [nch_total, 128] int64 global source ids per slot (pad 0)
      wd_list[q]: [128, nch_total] f32 dst_rel (pad 255)
    """
    row = np.asarray(edge_index[0]).astype(np.int64)
    col = np.asarray(edge_index[1]).astype(np.int64)
    groups = []
    for q in range(8):
        base = q * cfg.SHARD
        m = (col >= base) & (col < base + cfg.SHARD)
        r = row[m]
        c = col[m] - base
        t = c // P
        drel = c % P
        order = np.lexsort((r, drel, t))
        r, t, drel = r[order], t[order], drel[order]
        g = {}
        bounds = np.flatnonzero(np.append(True, t[1:] != t[:-1]))
        bounds = np.append(bounds, len(t))
        for j in range(len(bounds) - 1):
            s, e = int(bounds[j]), int(bounds[j + 1])
            g[int(t[s])] = (r[s:e], drel[s:e].astype(np.float32))
        groups.append(g)

    kmax = []
    for t in range(cfg.TILES):
        n = max(len(g.get(t, ((), ()))[0]) for g in groups)
        kmax.append(max(1, math.ceil(n / P)))
    chunk_meta = []
    for t in range(cfg.TILES):
        for j in range(kmax[t]):
            chunk_meta.append((t, j == 0, j == kmax[t] - 1))
    nch_total = len(chunk_meta)
    calls = [(s2, min(CPC, nch_total - s2)) for s2 in range(0, nch_total, CPC)]

    src_list, wd_list = [], []
    for q in range(8):
        srcs = np.zeros((nch_total, P), np.int64)
        wd = np.full((128, nch_total), 255.0, np.float32)
        c0 = 0
        for t in range(cfg.TILES):
            r, drel = groups[q].get(t, (np.zeros(0, np.int64),
                                        np.zeros(0, np.float32)))
            k = kmax[t]
            pr = np.zeros(k * P, np.int64)
            pd = np.full(k * P, 255.0, np.float32)
            pr[:len(r)] = r
            pd[:len(drel)] = drel
            srcs[c0:c0 + k] = pr.reshape(k, P)
            wd[:, c0:c0 + k] = pd.reshape(k, P).T
            c0 += k
        src_list.append(srcs)
        wd_list.append(wd)
    return calls, chunk_meta, src_list, wd_list


# ---------------------------------------------------------------- bass kernel


def build_nc(cfg, scheds, shapes):
    import concourse.bacc as bacc
    import concourse.bass as bass
    import concourse.tile as tile
    from concourse import mybir

    f32 = mybir.dt.float32
    bf16 = mybir.dt.bfloat16
    i16 = mybir.dt.int16
    TW = 2 * FEAT  # interleaved table width (256)

    (calls_x, meta_x), (calls_h, meta_h), (calls_s, meta_s) = scheds

    nc = bacc.Bacc("TRN2", target_bir_lowering=False, debug=False,
                   num_devices=8, num_swdge_queues=4)

    xg_d = nc.dram_tensor("xg", list(shapes["xg"]), bf16, kind="ExternalInput")
    wih_d = nc.dram_tensor("wih", list(shapes["wih"]), i16, kind="ExternalInput")
    wis_d = nc.dram_tensor("wis", list(shapes["wis"]), i16, kind="ExternalInput")
    wdx_d = nc.dram_tensor("wdx", list(shapes["wdx"]), bf16, kind="ExternalInput")
    wdh_d = nc.dram_tensor("wdh", list(shapes["wdh"]), bf16, kind="ExternalInput")
    wds_d = nc.dram_tensor("wds", list(shapes["wds"]), bf16, kind="ExternalInput")
    w_dr = [nc.dram_tensor(f"W{i+1}", [FEAT, FEAT if i < 2 else 1], f32,
                           kind="ExternalInput") for i in range(3)]
    b_dr = [nc.dram_tensor(f"b{i+1}", [FEAT if i < 2 else 1], f32,
                           kind="ExternalInput") for i in range(3)]
    gb_dr = [(nc.dram_tensor(f"gamma{i+1}", [FEAT], f32, kind="ExternalInput"),
              nc.dram_tensor(f"beta{i+1}", [FEAT], f32, kind="ExternalInput"))
             for i in range(2)]
    iota_p_d = nc.dram_tensor("iota_p", [P, P], bf16, kind="ExternalInput")
    ident_d = nc.dram_tensor("ident", [P, P], f32, kind="ExternalInput")
    out_d = nc.dram_tensor("out", [cfg.BATCH, cfg.SHARD], f32, kind="ExternalOutput")

    htab = nc.dram_tensor("htab0", [cfg.N, TW], bf16, kind="Internal",
                          addr_space="Shared")
    shard_out = nc.dram_tensor("shard_out0", [cfg.SHARD, TW], bf16,
                               kind="Internal")
    s_shard = nc.dram_tensor("s_shard", [SSH, 2], f32, kind="Internal")
    s_full = nc.dram_tensor("s_full", [S_PAD, 2], f32, kind="Internal")
    s_tab = nc.dram_tensor("s_tab", [S_PAD, SW], bf16, kind="Internal")
    stat_in = [nc.dram_tensor(f"stat_in{i}", [P, 2], f32, kind="Internal")
               for i in range(2)]
    stat_out = [nc.dram_tensor(f"stat_out{i}", [8 * P, 2], f32,
                               kind="Internal", addr_space="Shared")
                for i in range(2)]

    AluOp = mybir.AluOpType
    ActF = mybir.ActivationFunctionType

    def bcast_inner(ap, inner):
        return bass.AP(tensor=ap.tensor, offset=ap.offset,
                       ap=[list(ap.ap[0]), list(ap.ap[1]), [0, inner]])

    def bcast_rep(ap, reps):
        return bass.AP(tensor=ap.tensor, offset=ap.offset,
                       ap=[list(ap.ap[0]), [0, reps], list(ap.ap[1])])

    def re_ap(ap, free_dims, extra_off=0):
        """Same tensor/partition dim, custom free-dim APs."""
        return bass.AP(tensor=ap.tensor, offset=ap.offset + extra_off,
                       ap=[list(ap.ap[0])] + [list(d) for d in free_dims])

    with tile.TileContext(nc) as tc:
        with (
            tc.tile_pool(name="consts", bufs=1) as consts,
            tc.tile_pool(name="gw", bufs=9) as gwp,
            tc.tile_pool(name="ohp", bufs=6) as ohp,
            tc.tile_pool(name="aggp", bufs=2) as aggp,
            tc.tile_pool(name="hraw", bufs=1) as hrawp,
            tc.tile_pool(name="aglo", bufs=1) as aglop,
            tc.tile_pool(name="statp", bufs=1) as statp,
            tc.tile_pool(name="small", bufs=8) as small,
            tc.tile_pool(name="p2", bufs=6) as p2p,
            tc.tile_pool(name="sexp", bufs=2) as sexpp,
            tc.tile_pool(name="outp", bufs=1) as outp,
            tc.tile_pool(name="ps_agg", bufs=2, space="PSUM") as ps_agg,
            tc.tile_pool(name="ps_h", bufs=2, space="PSUM") as ps_h,
            tc.tile_pool(name="ps_t", bufs=2, space="PSUM") as ps_t,
        ):
            wih_sb = consts.tile(list(shapes["wih"]), i16, tag="wih")
            nc.sync.dma_start(out=wih_sb[:], in_=wih_d[:])
            wis_sb = consts.tile(list(shapes["wis"]), i16, tag="wis")
            nc.sync.dma_start(out=wis_sb[:], in_=wis_d[:])
            wdx_sb = consts.tile(list(shapes["wdx"]), bf16, tag="wdx")
            nc.sync.dma_start(out=wdx_sb[:], in_=wdx_d[:])
            wdh_sb = consts.tile(list(shapes["wdh"]), bf16, tag="wdh")
            nc.sync.dma_start(out=wdh_sb[:], in_=wdh_d[:])
            wds_sb = consts.tile(list(shapes["wds"]), bf16, tag="wds")
            nc.sync.dma_start(out=wds_sb[:], in_=wds_d[:])
            w_sb = []
            for i, wdr in enumerate(w_dr):
                t = consts.tile([P, FEAT if i < 2 else 1], f32, tag=f"w{i}")
                nc.sync.dma_start(out=t[:], in_=wdr[:])
                w_sb.append(t)
            b_sb = []
            for i in range(2):
                t = consts.tile([P, 1], f32, tag=f"b{i}")
                nc.sync.dma_start(out=t[:], in_=b_dr[i][:, None])
                b_sb.append(t)
            b3_2 = consts.tile([2, 1], f32, tag="b3_2")
            nc.sync.dma_start(out=b3_2[:], in_=b_dr[2][:].to_broadcast([2, 1]))
            gb_sb = []
            for i, (gd, bd) in enumerate(gb_dr):
                tg = consts.tile([P, 1], f32, tag=f"g{i}")
                nc.sync.dma_start(out=tg[:], in_=gd[:, None])
                tb = consts.tile([P, 1], f32, tag=f"be{i}")
                nc.sync.dma_start(out=tb[:], in_=bd[:, None])
                gb_sb.append((tg, tb))
            iota_p = consts.tile([P, P], bf16, tag="iota_p")
            nc.sync.dma_start(out=iota_p[:], in_=iota_p_d[:])
            ident = consts.tile([P, P], f32, tag="ident")
            nc.sync.dma_start(out=ident[:], in_=ident_d[:])
            eps_sb = consts.tile([P, 1], f32, tag="eps")
            nc.vector.memset(eps_sb[:], cfg.EPS)

            # ======================================================= bn tail
            def bn_scale_from_stats(layer, stat_t):
                """AR the per-core (mean, E[x^2]) and return (scal, shif)."""
                mv = small.tile([P, 2], f32, tag="mv")
                nc.vector.bn_aggr(out=mv[:], in_=stat_t[:, :, :])
                sloc = small.tile([P, 2], f32, tag="sloc")
                nc.vector.tensor_copy(out=sloc[:, 0:1], in_=mv[:, 0:1])
                nc.vector.tensor_tensor(
                    out=sloc[:, 1:2], in0=mv[:, 0:1], in1=mv[:, 0:1],
                    op=AluOp.mult,
                )
                nc.vector.tensor_add(
                    out=sloc[:, 1:2], in0=sloc[:, 1:2], in1=mv[:, 1:2]
                )
                nc.sync.dma_start(out=stat_in[layer][:], in_=sloc[:])
                nc.gpsimd.collective_compute(
                    "AllGather", AluOp.bypass,
                    replica_groups=[[0, 1, 2, 3, 4, 5, 6, 7]],
                    ins=[stat_in[layer][:]], outs=[stat_out[layer][:]],
                )
                s8 = small.tile([P, 8, 2], f32, tag="s8")
                nc.sync.dma_start(
                    out=s8[:],
                    in_=stat_out[layer][:, :].rearrange("(c p) b -> p c b",
                                                        p=P),
                )
                sglob = small.tile([P, 2], f32, tag="sglob")
                nc.vector.tensor_add(out=sglob[:], in0=s8[:, 0, :],
                                     in1=s8[:, 1, :])
                for c in range(2, 8):
                    nc.vector.tensor_add(out=sglob[:], in0=sglob[:],
                                         in1=s8[:, c, :])
                nc.scalar.mul(out=sglob[:], in_=sglob[:], mul=0.125)
                var = small.tile([P, 1], f32, tag="var")
                nc.vector.tensor_tensor(
                    out=var[:], in0=sglob[:, 0:1], in1=sglob[:, 0:1],
                    op=AluOp.mult,
                )
                nc.vector.tensor_sub(out=var[:], in0=sglob[:, 1:2], in1=var[:])
                rstd = small.tile([P, 1], f32, tag="rstd")
                nc.scalar.activation(out=rstd[:], in_=var[:], func=ActF.Sqrt,
                                     bias=eps_sb[:])
                nc.vector.reciprocal(out=rstd[:], in_=rstd[:])
                scal = small.tile([P, 1], f32, tag="scal")
                nc.vector.tensor_tensor(
                    out=scal[:], in0=gb_sb[layer][0][:], in1=rstd[:],
                    op=AluOp.mult,
                )
                shif = small.tile([P, 1], f32, tag="shif")
                nc.vector.tensor_tensor(
                    out=shif[:], in0=sglob[:, 0:1], in1=scal[:], op=AluOp.mult,
                )
                nc.vector.tensor_sub(out=shif[:], in0=gb_sb[layer][1][:],
                                     in1=shif[:])
                return scal, shif

            # ============================================ layer 0: streamed x
            hraw = [hrawp.tile([P, cfg.TILES * P], f32,
                               tag=f"hraw{b}", name=f"hraw{b}")
                    for b in range(2)]
            stat_t = statp.tile([P, 2 * cfg.TILES, 6], f32, tag="stats")

            agg_ps = None
            for ci, (c0, nch) in enumerate(calls_x):
                gt = gwp.tile([P, CPC, TW], bf16, tag="gw")
                eng = nc.sync if ci % 2 == 0 else nc.scalar
                eng.dma_start(
                    out=gt[:, :nch, :],
                    in_=xg_d[:, c0 * TW:(c0 + nch) * TW],
                )
                oh = ohp.tile([P, CPC * P], bf16, tag="oh")
                nc.vector.tensor_tensor(
                    out=oh[:, :nch * P],
                    in0=bcast_inner(wdx_sb[:, c0:c0 + nch], P),
                    in1=bcast_rep(iota_p[:], nch),
                    op=AluOp.is_equal,
                )
                for j in range(nch):
                    tt, first, last = meta_x[c0 + j]
                    if first:
                        agg_ps = [ps_agg.tile([P, P], f32, tag=f"agg{b}",
                                              name=f"agg{b}")
                                  for b in range(2)]
                    for b in range(2):
                        nc.tensor.matmul(
                            agg_ps[b][:, :],
                            lhsT=gt[:, j, b * FEAT:(b + 1) * FEAT],
                            rhs=oh[:, j * P:(j + 1) * P],
                            start=first, stop=last,
                        )
                    if last:
                        valid = cfg.VALID_LAST if tt == cfg.TILES - 1 else P
                        for b in range(2):
                            agg_sb = aggp.tile([P, P], f32, tag=f"aggsb{b}")
                            nc.scalar.activation(out=agg_sb[:],
                                                 in_=agg_ps[b][:],
                                                 func=ActF.Copy)
                            h_ps = ps_h.tile([P, P], f32, tag="hps")
                            nc.tensor.matmul(
                                h_ps[:], lhsT=w_sb[0][:],
                                rhs=agg_sb[:], start=True, stop=True,
                            )
                            nc.scalar.activation(
                                out=hraw[b][:, tt * P:tt * P + P],
                                in_=h_ps[:], func=ActF.Identity,
                                bias=b_sb[0][:],
                            )
                            nc.vector.bn_stats(
                                out=stat_t[:, 2 * tt + b, :],
                                in_=hraw[b][:, tt * P:tt * P + valid],
                            )

            # --------------------------- boundary 0: BN, pass-2, quarter AGs
            scal, shif = bn_scale_from_stats(0, stat_t)
            for b in range(2):
                nc.scalar.activation(
                    out=hraw[b][:], in_=hraw[b][:],
                    func=ActF.Relu, bias=shif[:], scale=scal[:],
                )
            for qi in range(2):
                t0, t1 = QT[2 * qi], QT[2 * qi + 2]
                for t in range(t0, t1):
                    valid = cfg.VALID_LAST if t == cfg.TILES - 1 else P
                    hrow2 = p2p.tile([P, TW], bf16, tag="hrow")
                    for b in range(2):
                        t_ps = ps_t.tile([P, P], f32, tag="tps")
                        nc.tensor.transpose(
                            out=t_ps[:], in_=hraw[b][:, t * P:(t + 1) * P],
                            identity=ident[:])
                        nc.vector.tensor_copy(
                            out=hrow2[:, b * FEAT:(b + 1) * FEAT],
                            in_=t_ps[:])
                    nc.sync.dma_start(
                        out=shard_out[t * P:t * P + valid, :],
                        in_=hrow2[:valid, :],
                    )
                r0 = QSTART[2 * qi]
                rsz = cfg.QSZ[2 * qi] + cfg.QSZ[2 * qi + 1]
                nc.gpsimd.collective_compute(
                    "AllGather", AluOp.bypass,
                    replica_groups=[[0, 1, 2, 3, 4, 5, 6, 7]],
                    ins=[shard_out[r0:r0 + rsz, :]],
                    outs=[htab[8 * r0:8 * r0 + 8 * rsz, :]],
                )

            # ====================================== layer 1: gather from htab
            hraw = [hrawp.tile([P, cfg.TILES * P], f32,
                               tag=f"hraw{b}", name=f"hraw{b}")
                    for b in range(2)]
            stat_t = statp.tile([P, 2 * cfg.TILES, 6], f32, tag="stats")
            agg_lo = [aglop.tile([P, cfg.TILES * P], bf16,
                                 tag=f"aglo{b}", name=f"aglo{b}")
                      for b in range(2)]
            qn = 0
            for (bb, c0, nch) in calls_h:
                gt = gwp.tile([P, CPC, TW], bf16, tag="gw")
                src = htab[0:cfg.HHALF, :] if bb == 0 else htab[cfg.HHALF:, :]
                nc.gpsimd.dma_gather(
                    gt[:, :nch, :], src,
                    wih_sb[:, c0 * 8:(c0 + nch) * 8],
                    nch * P, nch * P, TW,
                    queue_num=qn,
                )
                qn = (qn + 1) % 4
                oh = ohp.tile([P, CPC * P], bf16, tag="oh")
                nc.vector.tensor_tensor(
                    out=oh[:, :nch * P],
                    in0=bcast_inner(wdh_sb[:, c0:c0 + nch], P),
                    in1=bcast_rep(iota_p[:], nch),
                    op=AluOp.is_equal,
                )
                for j in range(nch):
                    tt, first, last = meta_h[c0 + j]
                    if first:
                        agg_ps = [ps_agg.tile([P, P], f32, tag=f"agg{b}",
                                              name=f"agg{b}")
                                  for b in range(2)]
                    for b in range(2):
                        nc.tensor.matmul(
                            agg_ps[b][:, :],
                            lhsT=gt[:, j, b * FEAT:(b + 1) * FEAT],
                            rhs=oh[:, j * P:(j + 1) * P],
                            start=first, stop=last,
                        )
                    if last and bb == 0:
                        for b in range(2):
                            nc.scalar.activation(
                                out=agg_lo[b][:, tt * P:(tt + 1) * P],
                                in_=agg_ps[b][:], func=ActF.Copy,
                            )
                    elif last and bb == 1:
                        valid = cfg.VALID_LAST if tt == cfg.TILES - 1 else P
                        for b in range(2):
                            agg_sb = aggp.tile([P, P], f32, tag=f"aggsb{b}")
                            nc.vector.tensor_add(
                                out=agg_sb[:], in0=agg_ps[b][:],
                                in1=agg_lo[b][:, tt * P:(tt + 1) * P],
                            )
                            h_ps = ps_h.tile([P, P], f32, tag="hps")
                            nc.tensor.matmul(
                                h_ps[:], lhsT=w_sb[1][:],
                                rhs=agg_sb[:], start=True, stop=True,
                            )
                            nc.vector.tensor_scalar_add(
                                out=hraw[b][:, tt * P:tt * P + P],
                                in0=h_ps[:], scalar1=b_sb[1][:],
                            )
                            nc.vector.bn_stats(
                                out=stat_t[:, 2 * tt + b, :],
                                in_=hraw[b][:, tt * P:tt * P + valid],
                            )

            # ---------------- boundary 1: BN, s = relu(bn(h2)) @ W3, s AG,
            # local expansion into the 256B-row s_tab
            scal, shif = bn_scale_from_stats(1, stat_t)
            for b in range(2):
                nc.scalar.activation(
                    out=hraw[b][:], in_=hraw[b][:],
                    func=ActF.Relu, bias=shif[:], scale=scal[:],
                )
            s_sb = outp.tile([P, cfg.TILES, 2], f32, tag="s_sb")
            for t in range(cfg.TILES):
                s_ps = ps_t.tile([P, P], f32, tag="tps")
                for b in range(2):
                    nc.tensor.matmul(s_ps[:, b:b + 1],
                                     lhsT=hraw[b][:, t * P:(t + 1) * P],
                                     rhs=w_sb[2][:],
                                     start=True, stop=True)
                nc.vector.tensor_copy(out=s_sb[:, t, :], in_=s_ps[:, 0:2])
            # p-major s_shard layout: position p*49+t = local node t*128+p,
            # so the store is one contiguous 392B run per partition and the
            # host remaps gather indices to match.
            nc.sync.dma_start(
                out=s_shard[:, :].rearrange("(p t) b -> p t b", t=cfg.TILES),
                in_=s_sb[:, :, :],
            )
            nc.gpsimd.collective_compute(
                "AllGather", AluOp.bypass,
                replica_groups=[[0, 1, 2, 3, 4, 5, 6, 7]],
                ins=[s_shard[:, :]], outs=[s_full[:, :]],
            )
            # expansion: s_tab row n = [s_b0(n) x128 | s_b1(n) x128] (bf16).
            # Slab maps partition p to a CONTIGUOUS row range so the store
            # is one big contiguous run per partition (fast DMA).
            for n0 in range(0, S_PAD, SLAB_N):
                rn = min(SLAB_R, (S_PAD - n0) // P)
                sexp = sexpp.tile([P, SLAB_R * SW], bf16, tag="sexp")
                sl = sexpp.tile([P, SLAB_R * 2], f32, tag="sl")
                nc.scalar.dma_start(
                    out=sl[:, :rn * 2],
                    in_=s_full[n0:n0 + rn * P, :].rearrange(
                        "(p r) b -> p (r b)", r=rn),
                )
                for b in range(2):
                    nc.vector.tensor_copy(
                        out=re_ap(sexp[:], [[SW, rn], [1, SW // 2]],
                                  extra_off=b * (SW // 2)),
                        in_=re_ap(sl[:], [[2, rn], [0, SW // 2]],
                                  extra_off=b),
                    )
                nc.sync.dma_start(
                    out=s_tab[n0:n0 + rn * P, :].rearrange(
                        "(p r) f -> p (r f)", r=rn),
                    in_=sexp[:, :rn * SW],
                )

            # ================================= layer 2: gather scalar s rows
            out_acc = outp.tile([2, cfg.TILES * P], f32, tag="out_acc")
            qn = 0
            for (bb, c0, nch) in calls_s:
                gt = gwp.tile([P, CPC, TW], bf16, tag="gw")
                src = s_tab[0:S_HALF, :] if bb == 0 else s_tab[S_HALF:, :]
                nc.gpsimd.dma_gather(
                    gt[:, :nch, :], src,
                    wis_sb[:, c0 * 8:(c0 + nch) * 8],
                    nch * P, nch * P, SW,
                    queue_num=qn,
                )
                qn = (qn + 1) % 4
                oh = ohp.tile([P, CPC * P], bf16, tag="oh")
                nc.vector.tensor_tensor(
                    out=oh[:, :nch * P],
                    in0=bcast_inner(wds_sb[:, c0:c0 + nch], P),
                    in1=bcast_rep(iota_p[:], nch),
                    op=AluOp.is_equal,
                )
                for j in range(nch):
                    tt, first, last = meta_s[c0 + j]
                    if first:
                        agg2 = ps_agg.tile([P, P], f32, tag="agg0",
                                           name="agg0")[0:2, :]
                    nc.tensor.matmul(
                        agg2[:, :],
                        lhsT=re_ap(gt[:], [[SW // 2, 2]], extra_off=j * SW),
                        rhs=oh[:, j * P:(j + 1) * P],
                        start=first, stop=last,
                    )
                    if last and bb == 0:
                        nc.scalar.activation(
                            out=out_acc[:, tt * P:(tt + 1) * P],
                            in_=agg2[:], func=ActF.Copy,
                        )
                    elif last and bb == 1:
                        nc.vector.tensor_add(
                            out=out_acc[:, tt * P:(tt + 1) * P],
                            in0=agg2[:],
                            in1=out_acc[:, tt * P:(tt + 1) * P],
                        )
            # bias + single output DMA
            nc.vector.tensor_scalar_add(
                out=out_acc[:, 0:cfg.SHARD], in0=out_acc[:, 0:cfg.SHARD],
                scalar1=b3_2[:],
            )
            nc.sync.dma_start(out=out_d[:, :], in_=out_acc[:, 0:cfg.SHARD])

    nc.compile()
    return nc


# ---------------------------------------------------------------- host + run


def run_gcn(cfg, inputs, trace=False):
    import ml_dtypes
    from concourse.bass_utils import run_bass_kernel_spmd

    x = np.asarray(inputs["x"], dtype=np.float32)
    edge_index = np.asarray(inputs["edge_index"])

    # half-block htab layout: [half0: cores 0-7 | half1: cores 0-7]
    qstart = np.array([0, QSTART[2], cfg.SHARD], dtype=np.int64)
    qoff = np.array([0, 8 * QSTART[2]], dtype=np.int64)
    qsz = np.array([QSTART[2], cfg.SHARD - QSTART[2]], dtype=np.int64)

    def remap_h(r):
        q, rr = r // cfg.SHARD, r % cfg.SHARD
        j = np.searchsorted(qstart, rr, side="right") - 1
        return qoff[j] + q * qsz[j] + (rr - qstart[j])

    calls_x, meta_x, src_list, wdx_list = build_schedule_x(cfg, edge_index)
    calls_h, meta_h, wih_list, wdh_list = build_schedule(cfg, edge_index,
                                                         remap_h,
                                                         half=cfg.HHALF)
    def remap_s(n):
        c, r = n // cfg.SHARD, n % cfg.SHARD
        return c * SSH + (r % P) * cfg.TILES + r // P

    calls_s, meta_s, wis_list, wds_list = build_schedule(cfg, edge_index,
                                                         remap_s,
                                                         half=S_HALF)

    # interleaved x table: row n = [x(b0,n,:) | x(b1,n,:)]; host pre-gather
    x_il = np.concatenate([x[0], x[1]], axis=1).astype(ml_dtypes.bfloat16)
    TW = 2 * FEAT
    nchx = len(meta_x)
    xg_list = []
    for q in range(8):
        g = x_il[src_list[q].reshape(-1)]  # [nchx*128, 256]
        g = g.reshape(nchx, P, TW).transpose(1, 0, 2).reshape(P, nchx * TW)
        xg_list.append(np.ascontiguousarray(g))

    shapes = {"xg": xg_list[0].shape, "wih": wih_list[0].shape,
              "wis": wis_list[0].shape, "wdx": wdx_list[0].shape,
              "wdh": wdh_list[0].shape, "wds": wds_list[0].shape}
    nc = build_nc(cfg, ((calls_x, meta_x), (calls_h, meta_h),
                        (calls_s, meta_s)), shapes)

    iota_p = np.tile(np.arange(P, dtype=np.float32), (P, 1))
    ident = np.eye(P, dtype=np.float32)
    common = {
        "W1": np.asarray(inputs["W1"], np.float32),
        "W2": np.asarray(inputs["W2"], np.float32),
        "W3": np.asarray(inputs["W3"], np.float32),
        "b1": np.asarray(inputs["b1"], np.float32),
        "b2": np.asarray(inputs["b2"], np.float32),
        "b3": np.asarray(inputs["b3"], np.float32),
        "gamma1": np.asarray(inputs["gamma1"], np.float32),
        "beta1": np.asarray(inputs["beta1"], np.float32),
        "gamma2": np.asarray(inputs["gamma2"], np.float32),
        "beta2": np.asarray(inputs["beta2"], np.float32),
        "iota_p": iota_p.astype(ml_dtypes.bfloat16),
        "ident": ident,
    }
    in_maps = []
    for c in range(8):
        m = dict(common)
        m["xg"] = xg_list[c]
        m["wih"] = wih_list[c]
        m["wis"] = wis_list[c]
        m["wdx"] = wdx_list[c].astype(ml_dtypes.bfloat16)
        m["wdh"] = wdh_list[c].astype(ml_dtypes.bfloat16)
        m["wds"] = wds_list[c].astype(ml_dtypes.bfloat16)
        in_maps.append(m)

    try:
        res = run_bass_kernel_spmd(nc, in_maps, core_ids=list(range(8)), trace=trace)
    except ModuleNotFoundError:
        res = run_bass_kernel_spmd(nc, in_maps, core_ids=list(range(8)), trace=False)
    out = np.empty((cfg.BATCH, cfg.N), np.float32)
    for c in range(8):
        out[:, c * cfg.SHARD:(c + 1) * cfg.SHARD] = res.results[c]["out"]
    return out, res


def kernel(**inputs) -> np.ndarray:
    cfg = Cfg()
    out, _ = run_gcn(cfg, inputs, trace=False)
    return out
